# revision 11
# baseline (speedup 1.0000x reference)
"""Trainium2 Bass kernel for nn_Attention (B=4, N=2048, D=1024, H=16, Hd=64).

Sharding: 8 cores = 4 batches x 2 head-groups; core c: batch c//2, heads
[(c%2)*8, (c%2)*8+8). Host sums the two partial projections per batch + bias.

v2 design (vs v1): keeps ScalarE (the 257us exp floor) saturated and pushes
TensorE below it via PE array tiling:
  - scores run as 4-way-concurrent 64x64 array tiles (2 heads x keys-lo/hi),
    ~216ns per [128 keys x 2x512 q] group (measured), using a 6-slice PSUM
    rotation [128,3072] so consecutive steps' chunks never WAR-block.
  - exp ops are [128,1024] PSUM->SBUF (1005ns measured) and always have
    inputs ready >=2 ops ahead (slice reuse distance 1.5 kt).
  - qkv/proj dense chains + pv (attn@V) all run as (128,64) column-paired
    MMs so per-step the PE sees only two tiling-mode switches.
  - softmax denominators: eacc adds on DVE (+ every 4th on GpSimd),
    ones-matmul partition reduce, reciprocal + gpsimd broadcast, with the
    normalize fused into the PSUM->SBUF drain (tensor_mul from PSUM).
  - U^T accumulates in a single PSUM bank [128,512] per (half, q-slice);
    the two q-slice passes per half are pipelined across half boundaries.
"""

import os
import sys
import types

import numpy as np

for _p in ("/opt/trn_rl_repo", "/root/.axon_site/_ro/trn_rl_repo"):
    if _p not in sys.path and os.path.isdir(_p):
        sys.path.append(_p)

import ml_dtypes  # noqa: E402

BF16 = ml_dtypes.bfloat16


def _install_ntff_shim():
    if "antenv.axon_hooks" in sys.modules:
        return
    mod = types.ModuleType("antenv.axon_hooks")
    mod._hook = None
    mod.set_axon_ntff_profile_hook = lambda h: setattr(mod, "_hook", h)
    mod.get_axon_ntff_profile_hook = lambda: mod._hook
    sys.modules["antenv.axon_hooks"] = mod
    try:
        import antenv

        antenv.axon_hooks = mod
    except ImportError:
        pass
    try:
        from trn_agent_boot.trn_boot import _ntff_profile_via_ctypes

        hook = _ntff_profile_via_ctypes("/opt/axon/libaxon_pjrt.so")
        if hook is not None:
            mod.set_axon_ntff_profile_hook(hook)
    except Exception:
        pass


_install_ntff_shim()

import concourse.bacc as bacc  # noqa: E402
import concourse.tile as tile  # noqa: E402
from concourse import mybir  # noqa: E402
import concourse.bass_utils as bass_utils  # noqa: E402

bass_utils.upload_artifacts = lambda tmpdir: tmpdir

F32 = mybir.dt.float32
BF = mybir.dt.bfloat16
EXP = mybir.ActivationFunctionType.Exp

N_CORES = 8
NT = 2048
D = 1024
HD = 64
SCALE = HD**-0.5

HALVES = [(0, 0), (1, 0), (2, 0), (3, 0), (0, 1), (1, 1), (2, 1), (3, 1)]


def _body(tc: "tile.TileContext", ctx, y, xT, wqk, wv, wp):
    nc = tc.nc

    wpool = ctx.enter_context(tc.tile_pool(name="wpool", bufs=1))
    e0pool = ctx.enter_context(tc.tile_pool(name="e0pool", bufs=8))
    e1pool = ctx.enter_context(tc.tile_pool(name="e1pool", bufs=17))
    eapool = ctx.enter_context(tc.tile_pool(name="eapool", bufs=2))
    spool = ctx.enter_context(tc.tile_pool(name="spool", bufs=2))
    recpool = ctx.enter_context(tc.tile_pool(name="recpool", bufs=4))
    opool = ctx.enter_context(tc.tile_pool(name="opool", bufs=2))
    psS = ctx.enter_context(tc.tile_pool(name="psS", bufs=1, space="PSUM"))
    psU = ctx.enter_context(tc.tile_pool(name="psU", bufs=1, space="PSUM"))
    psW = ctx.enter_context(tc.tile_pool(name="psW", bufs=1, space="PSUM"))

    # ---- persistent SBUF + ordered input DMAs ----------------------------
    xT_sb = [wpool.tile([128, NT], BF, tag=f"xT{i}", name=f"xT{i}") for i in range(8)]
    wqk_sb = [wpool.tile([128, 1024], BF, tag=f"wqk{i}", name=f"wqk{i}") for i in range(8)]
    wv_sb = [wpool.tile([128, 512], BF, tag=f"wv{i}", name=f"wv{i}") for i in range(8)]
    wp_sb = [wpool.tile([128, 1024], BF, tag=f"wp{i}", name=f"wp{i}") for i in range(4)]
    # earliest needs first: x tokens 0:1024, pair-0 qk weights, wv
    for ts in range(2):
        for i in range(8):
            nc.sync.dma_start(out=xT_sb[i][:, ts * 512:(ts + 1) * 512],
                              in_=xT[i * 128:(i + 1) * 128, ts * 512:(ts + 1) * 512])
    for i in range(8):
        nc.sync.dma_start(out=wqk_sb[i][:, 0:256], in_=wqk[i * 128:(i + 1) * 128, 0:256])
    for i in range(8):
        nc.sync.dma_start(out=wv_sb[i], in_=wv[i * 128:(i + 1) * 128, :])
    for ts in range(2, 4):
        for i in range(8):
            nc.sync.dma_start(out=xT_sb[i][:, ts * 512:(ts + 1) * 512],
                              in_=xT[i * 128:(i + 1) * 128, ts * 512:(ts + 1) * 512])
    for p in range(1, 4):
        for i in range(8):
            nc.sync.dma_start(out=wqk_sb[i][:, p * 256:(p + 1) * 256],
                              in_=wqk[i * 128:(i + 1) * 128, p * 256:(p + 1) * 256])
    for i in range(4):
        nc.sync.dma_start(out=wp_sb[i], in_=wp[i * 128:(i + 1) * 128, :])

    # qkT[2p] = Q features of pair p (rows: headA 0:64 | headB 64:128 hd),
    # qkT[2p+1] = K features of pair p.  cols = 2048 tokens.
    qkT = [wpool.tile([128, NT], BF, tag=f"qkT{f}", name=f"qkT{f}") for f in range(8)]
    v_sb = [wpool.tile([128, 512], BF, tag=f"v{t}", name=f"v{t}") for t in range(16)]
    uhat = [wpool.tile([128, NT], BF, tag=f"uh{p}", name=f"uh{p}") for p in range(4)]
    ones64 = wpool.tile([128, 64], BF, tag="ones64", name="ones64")
    nc.vector.memset(ones64, 1.0)

    stbig = psS.tile([128, 3072], F32, tag="stbig", name="stbig")  # 6 banks

    # ---- dense chains (M=128, (128,128) mode) ----------------------------
    def qk_chain(f, ts2):
        scr = psW.tile([128, 512], F32, tag="scr", name=f"qk{f}_{ts2}")
        for d in range(8):
            nc.tensor.matmul(scr, wqk_sb[d][:, f * 128:(f + 1) * 128],
                             xT_sb[d][:, ts2 * 512:(ts2 + 1) * 512],
                             start=(d == 0), stop=(d == 7))
        nc.vector.tensor_copy(out=qkT[f][:, ts2 * 512:(ts2 + 1) * 512], in_=scr)

    def v_chain(t):
        scr = psW.tile([128, 512], F32, tag="scr", name=f"v{t}")
        for d in range(8):
            nc.tensor.matmul(scr, xT_sb[d][:, t * 128:(t + 1) * 128], wv_sb[d],
                             start=(d == 0), stop=(d == 7))
        nc.vector.tensor_copy(out=v_sb[t], in_=scr)
        v_emitted[t] = True

    def proj_chain(qt, es):
        scr = psW.tile([128, 512], F32, tag="scr", name=f"pj{qt}_{es}")
        for c in range(4):
            nc.tensor.matmul(scr, uhat[c][:, qt * 128:(qt + 1) * 128],
                             wp_sb[c][:, es * 512:(es + 1) * 512],
                             start=(c == 0), stop=(c == 3))
        ot = opool.tile([128, 512], F32, tag="out", name=f"ot{qt}_{es}")
        nc.vector.tensor_copy(out=ot, in_=scr)
        nc.sync.dma_start(out=y[qt * 128:(qt + 1) * 128, es * 512:(es + 1) * 512], in_=ot)

    # ---- attention state --------------------------------------------------
    v_emitted = [False] * 16
    gc = [0]  # global chunk counter -> stbig slice rotation
    eS = {}  # (hidx, s, kt) -> exp'd scores [128, 1024] = [A | B]
    eacc = {}  # (hidx, s) -> accumulated E
    recs = {}  # (hidx, s) -> (recA, recB)
    passes = []  # pv pass FIFO: dicts

    def emit_scores_exps(hidx, p, h, kt):
        qk_q, qk_k = qkT[2 * p], qkT[2 * p + 1]
        sl = [(gc[0] + c) % 6 for c in range(4)]
        gc[0] += 4
        for s in range(2):  # chunk pair (A, B) for q-slice s
            q0 = h * 1024 + s * 512
            for head in range(2):
                r = slice(head * 64, head * 64 + 64)
                ch = stbig[:, sl[2 * s + head] * 512:(sl[2 * s + head] + 1) * 512]
                for lh in range(2):
                    k0 = kt * 128 + lh * 64
                    nc.tensor.matmul(ch[lh * 64:(lh + 1) * 64, :], qk_k[r, k0:k0 + 64],
                                     qk_q[r, q0:q0 + 512], start=True, stop=True)
        for s, pool in ((0, e0pool), (1, e1pool)):
            et = pool.tile([128, 1024], BF, tag=f"e{s}", name=f"e{hidx}_{s}_{kt}")
            a = sl[2 * s]
            nc.scalar.activation(out=et, in_=stbig[:, a * 512:(a + 2) * 512], func=EXP, scale=SCALE)
            eS[(hidx, s, kt)] = et
            # main accumulator on DVE; kt 5/9 go to a separate GpSimd
            # accumulator (merged by the sums matmuls) to stay off the chain
            if kt == 0:
                ea = eapool.tile([128, 1024], BF, tag=f"ea{s}", name=f"ea{hidx}_{s}")
                eacc[(hidx, s)] = ea
                nc.vector.tensor_copy(out=ea, in_=et)
            elif kt == 5:
                eg = eapool.tile([128, 1024], BF, tag=f"eg{s}", bufs=1, name=f"eg{hidx}_{s}")
                eacc[(hidx, s, "g")] = eg
                nc.gpsimd.tensor_copy(out=eg, in_=et)
            elif kt == 9:
                eg = eacc[(hidx, s, "g")]
                nc.gpsimd.tensor_add(out=eg, in0=eg, in1=et)
            else:
                nc.vector.tensor_add(out=eacc[(hidx, s)], in0=eacc[(hidx, s)], in1=et)

    def rec_chain(hidx, s):
        ea = eacc[(hidx, s)]
        eg = eacc[(hidx, s, "g")]
        sp = psW.tile([128, 512], F32, tag="scr", name=f"sums{hidx}_{s}")
        nc.tensor.matmul(sp[0:64, :], ones64, ea[:, 0:512], start=True, stop=False)
        nc.tensor.matmul(sp[0:64, :], ones64, eg[:, 0:512], start=False, stop=True)
        nc.tensor.matmul(sp[64:128, :], ones64, ea[:, 512:1024], start=True, stop=False)
        nc.tensor.matmul(sp[64:128, :], ones64, eg[:, 512:1024], start=False, stop=True)
        ss = spool.tile([128, 512], F32, tag="sums", name=f"ss{hidx}_{s}")
        nc.vector.tensor_copy(out=ss, in_=sp)
        out = []
        for hb in (0, 1):
            rsp = spool.tile([128, 4], F32, tag="rsp", name=f"rsp{hidx}_{s}_{hb}")
            row = ss[hb * 64:hb * 64 + 1, :].rearrange("p (a b) -> p a b", a=128)
            nc.gpsimd.dma_start(out=rsp, in_=row)
            rspr = spool.tile([128, 4], BF, tag="rspr", name=f"rspr{hidx}_{s}_{hb}")
            with nc.allow_low_precision(reason="bf16 softmax denominators are within tolerance"):
                nc.vector.reciprocal(out=rspr, in_=rsp)
            rrow = spool.tile([1, 512], BF, tag="rrow", bufs=1, name=f"rrow{hidx}_{s}_{hb}")
            nc.gpsimd.dma_start(out=rrow[0:1, :].rearrange("p (a b) -> p a b", a=128), in_=rspr)
            rec = recpool.tile([128, 512], BF, tag="rec", name=f"rec{hidx}_{s}_{hb}")
            nc.gpsimd.partition_broadcast(out_ap=rec[:, :], in_ap=rrow[0:1, :])
            out.append(rec)
        recs[(hidx, s)] = out

    def emit_norm(ps):
        # drain ut immediately (frees the single PSUM bank), then normalize
        # in place in SBUF once the reciprocal broadcast lands.
        p, h, s, hidx = ps["p"], ps["h"], ps["s"], ps["hidx"]
        recA, recB = recs[(hidx, s)]
        ucols = slice(h * 1024 + s * 512, h * 1024 + (s + 1) * 512)
        nc.vector.tensor_copy(out=uhat[p][:, ucols], in_=ps["ut"])
        nc.vector.tensor_mul(uhat[p][0:64, ucols], uhat[p][0:64, ucols], recA[0:64, :])
        nc.vector.tensor_mul(uhat[p][64:128, ucols], uhat[p][64:128, ucols], recB[64:128, :])

    def pump_pv(cur_hidx, cur_step, budget):
        while budget > 0 and passes:
            ps = passes[0]
            if ps["kt"] >= 16:
                if recs.get((ps["hidx"], ps["s"])) is None:
                    break  # rec not emitted yet (same-half): wait for end block
                emit_norm(ps)
                passes.pop(0)
                continue
            kt = ps["kt"]
            if ps["hidx"] == cur_hidx and kt > cur_step - 2:
                break
            if not v_emitted[kt]:
                break
            if ps["ut"] is None:
                ps["ut"] = psU.tile([128, 512], F32, tag="ut", name=f"ut{ps['hidx']}_{ps['s']}")
            e = eS.pop((ps["hidx"], ps["s"], kt))
            p = ps["p"]
            st, sp = (kt == 0), (kt == 15)
            nc.tensor.matmul(ps["ut"][0:64, :], v_sb[kt][:, p * 128:p * 128 + 64], e[:, 0:512], start=st, stop=sp)
            nc.tensor.matmul(ps["ut"][64:128, :], v_sb[kt][:, p * 128 + 64:(p + 1) * 128], e[:, 512:1024], start=st, stop=sp)
            ps["kt"] += 1
            budget -= 1

    # ---- dense filler schedule -------------------------------------------
    QK = lambda f, t: (lambda: qk_chain(f, t))
    V = lambda t: (lambda: v_chain(t))
    PJ = lambda qt, es: (lambda: proj_chain(qt, es))
    fillers = [
        # p0h0: v stream + K-p0 incremental + pair-1 features early
        [V(2), QK(1, 1), V(3), QK(2, 0), V(4), QK(1, 2), V(5), QK(2, 1),
         V(6), QK(1, 3), V(7), QK(3, 0), V(8), V(9), V(10), V(11)],
        # p1h0
        [V(12), V(13), QK(3, 1), V(14), QK(3, 2), V(15), QK(3, 3), QK(4, 0),
         QK(4, 1), QK(5, 0), QK(5, 1)],
        # p2h0
        [QK(5, 2), QK(5, 3), QK(6, 0), QK(6, 1), QK(7, 0), QK(7, 1)],
        # p3h0
        [QK(7, 2), QK(7, 3), QK(0, 2), QK(0, 3), QK(2, 2), QK(2, 3)],
        # p0h1
        [QK(4, 2), QK(4, 3), QK(6, 2), QK(6, 3)],
        # p1h1 .. p3h1: proj for h0 q-tiles
        [PJ(qt, es) for qt in range(0, 3) for es in range(2)],
        [PJ(qt, es) for qt in range(3, 6) for es in range(2)],
        [PJ(qt, es) for qt in range(6, 8) for es in range(2)],
    ]

    # ---- lead-in ----------------------------------------------------------
    qk_chain(0, 0)
    qk_chain(0, 1)
    qk_chain(1, 0)
    v_chain(0)
    v_chain(1)

    # ---- main loop --------------------------------------------------------
    for hidx, (p, h) in enumerate(HALVES):
        fl = list(fillers[hidx])
        for kt in range(16):
            emit_scores_exps(hidx, p, h, kt)
            if fl:
                fl.pop(0)()
            if kt == 8:
                passes.append({"hidx": hidx, "p": p, "h": h, "s": 0, "kt": 0, "ut": None})
            pump_pv(hidx, kt, 3)
        rec_chain(hidx, 0)
        rec_chain(hidx, 1)
        passes.append({"hidx": hidx, "p": p, "h": h, "s": 1, "kt": 0, "ut": None})

    # ---- tail -------------------------------------------------------------
    guard = 0
    while passes and guard < 200:
        pump_pv(-1, 99, 4)
        guard += 1
    for qt in range(8, 16):
        for es in range(2):
            proj_chain(qt, es)


_NC_CACHE = {}


def _build_nc():
    if "nc" in _NC_CACHE:
        return _NC_CACHE["nc"]
    nc = bacc.Bacc("TRN2", target_bir_lowering=False, debug=False, num_devices=N_CORES)
    xT = nc.dram_tensor("xT", [D, NT], BF, kind="ExternalInput").ap()
    wqk = nc.dram_tensor("wqk", [D, 1024], BF, kind="ExternalInput").ap()
    wv = nc.dram_tensor("wv", [D, 512], BF, kind="ExternalInput").ap()
    wp = nc.dram_tensor("wp", [512, 1024], BF, kind="ExternalInput").ap()
    y = nc.dram_tensor("y", [NT, 1024], F32, kind="ExternalOutput").ap()
    from contextlib import ExitStack

    with tile.TileContext(nc) as tc, ExitStack() as ctx:
        _body(tc, ctx, y, xT, wqk, wv, wp)
    nc.compile()
    _NC_CACHE["nc"] = nc
    return nc


def _prepare_in_maps(x, W_qkv, W_proj):
    x = np.asarray(x, dtype=np.float32)
    W_qkv = np.asarray(W_qkv, dtype=np.float32)
    W_proj = np.asarray(W_proj, dtype=np.float32)
    in_maps = []
    for c in range(N_CORES):
        b, hg = divmod(c, 2)
        cs = slice(hg * 512, (hg + 1) * 512)
        xTc = np.ascontiguousarray(x[b].T).astype(BF16)
        Qc = W_qkv[:, 0:1024][:, cs]
        Kc = W_qkv[:, 1024:2048][:, cs]
        # per-pair interleave: [Q-pair0 | K-pair0 | Q-pair1 | K-pair1 | ...]
        blocks = []
        for p in range(4):
            blocks.append(Qc[:, p * 128:(p + 1) * 128])
            blocks.append(Kc[:, p * 128:(p + 1) * 128])
        wqk = np.ascontiguousarray(np.concatenate(blocks, axis=1)).astype(BF16)
        wv = np.ascontiguousarray(W_qkv[:, 2048:3072][:, cs]).astype(BF16)
        wp = np.ascontiguousarray(W_proj[cs, :]).astype(BF16)
        in_maps.append({"xT": xTc, "wqk": wqk, "wv": wv, "wp": wp})
    return in_maps


def _run(x, W_qkv, W_proj, b_proj, trace=False):
    nc = _build_nc()
    in_maps = _prepare_in_maps(x, W_qkv, W_proj)
    res = bass_utils.run_bass_kernel_spmd(
        nc, in_maps, core_ids=list(range(N_CORES)), trace=trace
    )
    b_proj = np.asarray(b_proj, dtype=np.float32)
    y = np.empty((4, NT, D), dtype=np.float32)
    for b in range(4):
        y[b] = res.results[2 * b]["y"] + res.results[2 * b + 1]["y"] + b_proj[None, :]
    return y, res


def kernel(x, W_qkv, W_proj, b_proj):
    y, _ = _run(x, W_qkv, W_proj, b_proj, trace=False)
    return y


# revision 13
# speedup vs baseline: 1.2877x; 1.2877x over previous
"""Trainium2 Bass kernel for nn_Attention (B=4, N=2048, D=1024, H=16, Hd=64).

Sharding: 8 cores = 4 batches x 2 head-groups; core c: batch c//2, heads
[(c%2)*8, (c%2)*8+8). Host sums the two partial projections per batch + bias.

v2 design (vs v1): keeps ScalarE (the 257us exp floor) saturated and pushes
TensorE below it via PE array tiling:
  - scores run as 4-way-concurrent 64x64 array tiles (2 heads x keys-lo/hi),
    ~216ns per [128 keys x 2x512 q] group (measured), using a 6-slice PSUM
    rotation [128,3072] so consecutive steps' chunks never WAR-block.
  - exp ops are [128,1024] PSUM->SBUF (1005ns measured) and always have
    inputs ready >=2 ops ahead (slice reuse distance 1.5 kt).
  - qkv/proj dense chains + pv (attn@V) all run as (128,64) column-paired
    MMs so per-step the PE sees only two tiling-mode switches.
  - softmax denominators: eacc adds on DVE (+ every 4th on GpSimd),
    ones-matmul partition reduce, reciprocal + gpsimd broadcast, with the
    normalize fused into the PSUM->SBUF drain (tensor_mul from PSUM).
  - U^T accumulates in a single PSUM bank [128,512] per (half, q-slice);
    the two q-slice passes per half are pipelined across half boundaries.
"""

import os
import sys
import types

import numpy as np

for _p in ("/opt/trn_rl_repo", "/root/.axon_site/_ro/trn_rl_repo"):
    if _p not in sys.path and os.path.isdir(_p):
        sys.path.append(_p)

import ml_dtypes  # noqa: E402

BF16 = ml_dtypes.bfloat16


def _install_ntff_shim():
    if "antenv.axon_hooks" in sys.modules:
        return
    mod = types.ModuleType("antenv.axon_hooks")
    mod._hook = None
    mod.set_axon_ntff_profile_hook = lambda h: setattr(mod, "_hook", h)
    mod.get_axon_ntff_profile_hook = lambda: mod._hook
    sys.modules["antenv.axon_hooks"] = mod
    try:
        import antenv

        antenv.axon_hooks = mod
    except ImportError:
        pass
    try:
        from trn_agent_boot.trn_boot import _ntff_profile_via_ctypes

        hook = _ntff_profile_via_ctypes("/opt/axon/libaxon_pjrt.so")
        if hook is not None:
            mod.set_axon_ntff_profile_hook(hook)
    except Exception:
        pass


_install_ntff_shim()

import concourse.bacc as bacc  # noqa: E402
import concourse.tile as tile  # noqa: E402
from concourse import mybir  # noqa: E402
import concourse.bass_utils as bass_utils  # noqa: E402

bass_utils.upload_artifacts = lambda tmpdir: tmpdir

F32 = mybir.dt.float32
BF = mybir.dt.bfloat16
EXP = mybir.ActivationFunctionType.Exp

N_CORES = 8
NT = 2048
D = 1024
HD = 64
SCALE = HD**-0.5

HALVES = [(0, 0), (1, 0), (2, 0), (3, 0), (0, 1), (1, 1), (2, 1), (3, 1)]


def _body(tc: "tile.TileContext", ctx, y, xT, wqk, wv, wp):
    nc = tc.nc

    wpool = ctx.enter_context(tc.tile_pool(name="wpool", bufs=1))
    e0pool = ctx.enter_context(tc.tile_pool(name="e0pool", bufs=8))
    e1pool = ctx.enter_context(tc.tile_pool(name="e1pool", bufs=17))
    eapool = ctx.enter_context(tc.tile_pool(name="eapool", bufs=2))
    spool = ctx.enter_context(tc.tile_pool(name="spool", bufs=2))
    recpool = ctx.enter_context(tc.tile_pool(name="recpool", bufs=4))
    opool = ctx.enter_context(tc.tile_pool(name="opool", bufs=2))
    psS = ctx.enter_context(tc.tile_pool(name="psS", bufs=1, space="PSUM"))
    psU = ctx.enter_context(tc.tile_pool(name="psU", bufs=1, space="PSUM"))
    psW = ctx.enter_context(tc.tile_pool(name="psW", bufs=1, space="PSUM"))

    # ---- persistent SBUF + ordered input DMAs ----------------------------
    xT_sb = [wpool.tile([128, NT], BF, tag=f"xT{i}", name=f"xT{i}") for i in range(8)]
    wqk_sb = [wpool.tile([128, 1024], BF, tag=f"wqk{i}", name=f"wqk{i}") for i in range(8)]
    wv_sb = [wpool.tile([128, 512], BF, tag=f"wv{i}", name=f"wv{i}") for i in range(8)]
    wp_sb = [wpool.tile([128, 1024], BF, tag=f"wp{i}", name=f"wp{i}") for i in range(4)]
    # earliest needs first: x tokens 0:1024, pair-0 qk weights, wv
    for ts in range(2):
        for i in range(8):
            nc.sync.dma_start(out=xT_sb[i][:, ts * 512:(ts + 1) * 512],
                              in_=xT[i * 128:(i + 1) * 128, ts * 512:(ts + 1) * 512])
    for i in range(8):
        nc.sync.dma_start(out=wqk_sb[i][:, 0:256], in_=wqk[i * 128:(i + 1) * 128, 0:256])
    for i in range(8):
        nc.sync.dma_start(out=wv_sb[i], in_=wv[i * 128:(i + 1) * 128, :])
    for ts in range(2, 4):
        for i in range(8):
            nc.sync.dma_start(out=xT_sb[i][:, ts * 512:(ts + 1) * 512],
                              in_=xT[i * 128:(i + 1) * 128, ts * 512:(ts + 1) * 512])
    for p in range(1, 4):
        for i in range(8):
            nc.sync.dma_start(out=wqk_sb[i][:, p * 256:(p + 1) * 256],
                              in_=wqk[i * 128:(i + 1) * 128, p * 256:(p + 1) * 256])
    for i in range(4):
        nc.sync.dma_start(out=wp_sb[i], in_=wp[i * 128:(i + 1) * 128, :])

    # qkT[2p] = Q features of pair p (rows: headA 0:64 | headB 64:128 hd),
    # qkT[2p+1] = K features of pair p.  cols = 2048 tokens.
    qkT = [wpool.tile([128, NT], BF, tag=f"qkT{f}", name=f"qkT{f}") for f in range(8)]
    v_sb = [wpool.tile([128, 512], BF, tag=f"v{t}", name=f"v{t}") for t in range(16)]
    uhat = [wpool.tile([128, NT], BF, tag=f"uh{p}", name=f"uh{p}") for p in range(4)]
    ones64 = wpool.tile([128, 64], BF, tag="ones64", name="ones64")
    nc.vector.memset(ones64, 1.0)

    stbig = psS.tile([128, 3072], F32, tag="stbig", name="stbig")  # 6 banks

    # ---- dense chains (M=128, (128,128) mode) ----------------------------
    def qk_chain(f, ts2):
        scr = psW.tile([128, 512], F32, tag="scr", name=f"qk{f}_{ts2}")
        for d in range(8):
            nc.tensor.matmul(scr, wqk_sb[d][:, f * 128:(f + 1) * 128],
                             xT_sb[d][:, ts2 * 512:(ts2 + 1) * 512],
                             start=(d == 0), stop=(d == 7))
        nc.vector.tensor_copy(out=qkT[f][:, ts2 * 512:(ts2 + 1) * 512], in_=scr)

    def v_chain(t):
        scr = psW.tile([128, 512], F32, tag="scr", name=f"v{t}")
        for d in range(8):
            nc.tensor.matmul(scr, xT_sb[d][:, t * 128:(t + 1) * 128], wv_sb[d],
                             start=(d == 0), stop=(d == 7))
        nc.vector.tensor_copy(out=v_sb[t], in_=scr)
        v_emitted[t] = True

    def proj_chain(qt, es):
        scr = psW.tile([128, 512], F32, tag="scr", name=f"pj{qt}_{es}")
        for c in range(4):
            nc.tensor.matmul(scr, uhat[c][:, qt * 128:(qt + 1) * 128],
                             wp_sb[c][:, es * 512:(es + 1) * 512],
                             start=(c == 0), stop=(c == 3))
        ot = opool.tile([128, 512], F32, tag="out", name=f"ot{qt}_{es}")
        nc.vector.tensor_copy(out=ot, in_=scr)
        nc.sync.dma_start(out=y[qt * 128:(qt + 1) * 128, es * 512:(es + 1) * 512], in_=ot)

    # ---- attention state --------------------------------------------------
    v_emitted = [False] * 16
    gc = [0]  # global chunk counter -> stbig slice rotation
    eS = {}  # (hidx, s, kt) -> exp'd scores [128, 1024] = [A | B]
    eacc = {}  # (hidx, s) -> accumulated E
    recs = {}  # (hidx, s) -> (recA, recB)
    passes = []  # pv pass FIFO: dicts

    def emit_scores_exps(hidx, p, h, kt):
        qk_q, qk_k = qkT[2 * p], qkT[2 * p + 1]
        sl = [(gc[0] + c) % 6 for c in range(4)]
        gc[0] += 4
        for s in range(2):  # chunk pair (A, B) for q-slice s
            q0 = h * 1024 + s * 512
            for head in range(2):
                r = slice(head * 64, head * 64 + 64)
                ch = stbig[:, sl[2 * s + head] * 512:(sl[2 * s + head] + 1) * 512]
                for lh in range(2):
                    k0 = kt * 128 + lh * 64
                    nc.tensor.matmul(ch[lh * 64:(lh + 1) * 64, :], qk_k[r, k0:k0 + 64],
                                     qk_q[r, q0:q0 + 512], start=True, stop=True)
        for s, pool in ((0, e0pool), (1, e1pool)):
            et = pool.tile([128, 1024], BF, tag=f"e{s}", name=f"e{hidx}_{s}_{kt}")
            a = sl[2 * s]
            nc.scalar.activation(out=et, in_=stbig[:, a * 512:(a + 2) * 512], func=EXP, scale=SCALE)
            eS[(hidx, s, kt)] = et
            if kt == 0:
                ea = eapool.tile([128, 1024], BF, tag=f"ea{s}", name=f"ea{hidx}_{s}")
                eacc[(hidx, s)] = ea
                nc.vector.tensor_copy(out=ea, in_=et)
            else:
                nc.vector.tensor_add(out=eacc[(hidx, s)], in0=eacc[(hidx, s)], in1=et)

    def rec_chain(hidx, s):
        ea = eacc[(hidx, s)]
        sp = psW.tile([128, 512], F32, tag="scr", name=f"sums{hidx}_{s}")
        nc.tensor.matmul(sp[0:64, :], ones64, ea[:, 0:512], start=True, stop=True)
        nc.tensor.matmul(sp[64:128, :], ones64, ea[:, 512:1024], start=True, stop=True)
        ss = spool.tile([128, 512], F32, tag="sums", name=f"ss{hidx}_{s}")
        nc.vector.tensor_copy(out=ss, in_=sp)
        out = []
        for hb in (0, 1):
            rsp = spool.tile([128, 4], F32, tag="rsp", name=f"rsp{hidx}_{s}_{hb}")
            row = ss[hb * 64:hb * 64 + 1, :].rearrange("p (a b) -> p a b", a=128)
            nc.gpsimd.dma_start(out=rsp, in_=row)
            rspr = spool.tile([128, 4], BF, tag="rspr", name=f"rspr{hidx}_{s}_{hb}")
            with nc.allow_low_precision(reason="bf16 softmax denominators are within tolerance"):
                nc.vector.reciprocal(out=rspr, in_=rsp)
            rrow = spool.tile([1, 512], BF, tag="rrow", bufs=1, name=f"rrow{hidx}_{s}_{hb}")
            nc.gpsimd.dma_start(out=rrow[0:1, :].rearrange("p (a b) -> p a b", a=128), in_=rspr)
            rec = recpool.tile([128, 512], BF, tag="rec", name=f"rec{hidx}_{s}_{hb}")
            nc.gpsimd.partition_broadcast(out_ap=rec[:, :], in_ap=rrow[0:1, :])
            out.append(rec)
        recs[(hidx, s)] = out

    def emit_norm(ps):
        # drain ut immediately (frees the single PSUM bank), then normalize
        # in place in SBUF once the reciprocal broadcast lands.
        p, h, s, hidx = ps["p"], ps["h"], ps["s"], ps["hidx"]
        recA, recB = recs[(hidx, s)]
        ucols = slice(h * 1024 + s * 512, h * 1024 + (s + 1) * 512)
        nc.vector.tensor_copy(out=uhat[p][:, ucols], in_=ps["ut"])
        nc.vector.tensor_mul(uhat[p][0:64, ucols], uhat[p][0:64, ucols], recA[0:64, :])
        nc.vector.tensor_mul(uhat[p][64:128, ucols], uhat[p][64:128, ucols], recB[64:128, :])

    def pump_pv(cur_hidx, cur_step, budget):
        while budget > 0 and passes:
            ps = passes[0]
            if ps["kt"] >= 16:
                if recs.get((ps["hidx"], ps["s"])) is None:
                    break  # rec not emitted yet (same-half): wait for end block
                emit_norm(ps)
                passes.pop(0)
                continue
            kt = ps["kt"]
            if ps["hidx"] == cur_hidx and kt > cur_step - 2:
                break
            if not v_emitted[kt]:
                break
            if ps["ut"] is None:
                ps["ut"] = psU.tile([128, 512], F32, tag="ut", name=f"ut{ps['hidx']}_{ps['s']}")
            e = eS.pop((ps["hidx"], ps["s"], kt))
            p = ps["p"]
            st, sp = (kt == 0), (kt == 15)
            nc.tensor.matmul(ps["ut"][0:64, :], v_sb[kt][:, p * 128:p * 128 + 64], e[:, 0:512], start=st, stop=sp)
            nc.tensor.matmul(ps["ut"][64:128, :], v_sb[kt][:, p * 128 + 64:(p + 1) * 128], e[:, 512:1024], start=st, stop=sp)
            ps["kt"] += 1
            budget -= 1

    # ---- dense filler schedule -------------------------------------------
    QK = lambda f, t: (lambda: qk_chain(f, t))
    V = lambda t: (lambda: v_chain(t))
    PJ = lambda qt, es: (lambda: proj_chain(qt, es))
    fillers = [
        # p0h0: v stream + K-p0 incremental + pair-1 features early
        [V(2), QK(1, 1), V(3), QK(2, 0), V(4), QK(1, 2), V(5), QK(2, 1),
         V(6), QK(1, 3), V(7), QK(3, 0), V(8), V(9), V(10), V(11)],
        # p1h0
        [V(12), V(13), QK(3, 1), V(14), QK(3, 2), V(15), QK(3, 3), QK(4, 0),
         QK(4, 1), QK(5, 0), QK(5, 1)],
        # p2h0
        [QK(5, 2), QK(5, 3), QK(6, 0), QK(6, 1), QK(7, 0), QK(7, 1)],
        # p3h0
        [QK(7, 2), QK(7, 3), QK(0, 2), QK(0, 3), QK(2, 2), QK(2, 3)],
        # p0h1
        [QK(4, 2), QK(4, 3), QK(6, 2), QK(6, 3)],
        # p1h1 .. p3h1: proj for h0 q-tiles
        [PJ(qt, es) for qt in range(0, 3) for es in range(2)],
        [PJ(qt, es) for qt in range(3, 6) for es in range(2)],
        [PJ(qt, es) for qt in range(6, 8) for es in range(2)],
    ]

    # ---- lead-in ----------------------------------------------------------
    qk_chain(0, 0)
    qk_chain(0, 1)
    qk_chain(1, 0)
    v_chain(0)
    v_chain(1)

    # ---- main loop --------------------------------------------------------
    for hidx, (p, h) in enumerate(HALVES):
        fl = list(fillers[hidx])
        for kt in range(16):
            if fl:
                fl.pop(0)()
            emit_scores_exps(hidx, p, h, kt)
            if kt == 8:
                passes.append({"hidx": hidx, "p": p, "h": h, "s": 0, "kt": 0, "ut": None})
            pump_pv(hidx, kt, 3)
        rec_chain(hidx, 0)
        rec_chain(hidx, 1)
        passes.append({"hidx": hidx, "p": p, "h": h, "s": 1, "kt": 0, "ut": None})

    # ---- tail -------------------------------------------------------------
    guard = 0
    while passes and guard < 200:
        pump_pv(-1, 99, 4)
        guard += 1
    for qt in range(8, 16):
        for es in range(2):
            proj_chain(qt, es)


_NC_CACHE = {}


def _build_nc():
    if "nc" in _NC_CACHE:
        return _NC_CACHE["nc"]
    nc = bacc.Bacc("TRN2", target_bir_lowering=False, debug=False, num_devices=N_CORES)
    xT = nc.dram_tensor("xT", [D, NT], BF, kind="ExternalInput").ap()
    wqk = nc.dram_tensor("wqk", [D, 1024], BF, kind="ExternalInput").ap()
    wv = nc.dram_tensor("wv", [D, 512], BF, kind="ExternalInput").ap()
    wp = nc.dram_tensor("wp", [512, 1024], BF, kind="ExternalInput").ap()
    y = nc.dram_tensor("y", [NT, 1024], F32, kind="ExternalOutput").ap()
    from contextlib import ExitStack

    with tile.TileContext(nc) as tc, ExitStack() as ctx:
        _body(tc, ctx, y, xT, wqk, wv, wp)
    nc.compile()
    _NC_CACHE["nc"] = nc
    return nc


def _prepare_in_maps(x, W_qkv, W_proj):
    x = np.asarray(x, dtype=np.float32)
    W_qkv = np.asarray(W_qkv, dtype=np.float32)
    W_proj = np.asarray(W_proj, dtype=np.float32)
    in_maps = []
    for c in range(N_CORES):
        b, hg = divmod(c, 2)
        cs = slice(hg * 512, (hg + 1) * 512)
        xTc = np.ascontiguousarray(x[b].T).astype(BF16)
        Qc = W_qkv[:, 0:1024][:, cs]
        Kc = W_qkv[:, 1024:2048][:, cs]
        # per-pair interleave: [Q-pair0 | K-pair0 | Q-pair1 | K-pair1 | ...]
        blocks = []
        for p in range(4):
            blocks.append(Qc[:, p * 128:(p + 1) * 128])
            blocks.append(Kc[:, p * 128:(p + 1) * 128])
        wqk = np.ascontiguousarray(np.concatenate(blocks, axis=1)).astype(BF16)
        wv = np.ascontiguousarray(W_qkv[:, 2048:3072][:, cs]).astype(BF16)
        wp = np.ascontiguousarray(W_proj[cs, :]).astype(BF16)
        in_maps.append({"xT": xTc, "wqk": wqk, "wv": wv, "wp": wp})
    return in_maps


def _run(x, W_qkv, W_proj, b_proj, trace=False):
    nc = _build_nc()
    in_maps = _prepare_in_maps(x, W_qkv, W_proj)
    res = bass_utils.run_bass_kernel_spmd(
        nc, in_maps, core_ids=list(range(N_CORES)), trace=trace
    )
    b_proj = np.asarray(b_proj, dtype=np.float32)
    y = np.empty((4, NT, D), dtype=np.float32)
    for b in range(4):
        y[b] = res.results[2 * b]["y"] + res.results[2 * b + 1]["y"] + b_proj[None, :]
    return y, res


def kernel(x, W_qkv, W_proj, b_proj):
    y, _ = _run(x, W_qkv, W_proj, b_proj, trace=False)
    return y


# revision 14
# speedup vs baseline: 1.5018x; 1.1663x over previous
"""Trainium2 Bass kernel for nn_Attention (B=4, N=2048, D=1024, H=16, Hd=64).

Sharding: 8 cores = 4 batches x 2 head-groups; core c: batch c//2, heads
[(c%2)*8, (c%2)*8+8). Host sums the two partial projections per batch + bias.

v2 design (vs v1): keeps ScalarE (the 257us exp floor) saturated and pushes
TensorE below it via PE array tiling:
  - scores run as 4-way-concurrent 64x64 array tiles (2 heads x keys-lo/hi),
    ~216ns per [128 keys x 2x512 q] group (measured), using a 6-slice PSUM
    rotation [128,3072] so consecutive steps' chunks never WAR-block.
  - exp ops are [128,1024] PSUM->SBUF (1005ns measured) and always have
    inputs ready >=2 ops ahead (slice reuse distance 1.5 kt).
  - qkv/proj dense chains + pv (attn@V) all run as (128,64) column-paired
    MMs so per-step the PE sees only two tiling-mode switches.
  - softmax denominators: eacc adds on DVE (+ every 4th on GpSimd),
    ones-matmul partition reduce, reciprocal + gpsimd broadcast, with the
    normalize fused into the PSUM->SBUF drain (tensor_mul from PSUM).
  - U^T accumulates in a single PSUM bank [128,512] per (half, q-slice);
    the two q-slice passes per half are pipelined across half boundaries.
"""

import os
import sys
import types

import numpy as np

for _p in ("/opt/trn_rl_repo", "/root/.axon_site/_ro/trn_rl_repo"):
    if _p not in sys.path and os.path.isdir(_p):
        sys.path.append(_p)

import ml_dtypes  # noqa: E402

BF16 = ml_dtypes.bfloat16


def _install_ntff_shim():
    if "antenv.axon_hooks" in sys.modules:
        return
    mod = types.ModuleType("antenv.axon_hooks")
    mod._hook = None
    mod.set_axon_ntff_profile_hook = lambda h: setattr(mod, "_hook", h)
    mod.get_axon_ntff_profile_hook = lambda: mod._hook
    sys.modules["antenv.axon_hooks"] = mod
    try:
        import antenv

        antenv.axon_hooks = mod
    except ImportError:
        pass
    try:
        from trn_agent_boot.trn_boot import _ntff_profile_via_ctypes

        hook = _ntff_profile_via_ctypes("/opt/axon/libaxon_pjrt.so")
        if hook is not None:
            mod.set_axon_ntff_profile_hook(hook)
    except Exception:
        pass


_install_ntff_shim()

import concourse.bacc as bacc  # noqa: E402
import concourse.tile as tile  # noqa: E402
from concourse import mybir  # noqa: E402
import concourse.bass_utils as bass_utils  # noqa: E402

bass_utils.upload_artifacts = lambda tmpdir: tmpdir

F32 = mybir.dt.float32
BF = mybir.dt.bfloat16
EXP = mybir.ActivationFunctionType.Exp

N_CORES = 8
NT = 2048
D = 1024
HD = 64
SCALE = HD**-0.5

HALVES = [(0, 0), (1, 0), (2, 0), (3, 0), (0, 1), (1, 1), (2, 1), (3, 1)]


def _body(tc: "tile.TileContext", ctx, y, xT, wqk, wv, wp):
    nc = tc.nc

    wpool = ctx.enter_context(tc.tile_pool(name="wpool", bufs=1))
    e0pool = ctx.enter_context(tc.tile_pool(name="e0pool", bufs=8))
    e1pool = ctx.enter_context(tc.tile_pool(name="e1pool", bufs=17))
    eapool = ctx.enter_context(tc.tile_pool(name="eapool", bufs=2))
    spool = ctx.enter_context(tc.tile_pool(name="spool", bufs=2))
    recpool = ctx.enter_context(tc.tile_pool(name="recpool", bufs=4))
    opool = ctx.enter_context(tc.tile_pool(name="opool", bufs=2))
    psS = ctx.enter_context(tc.tile_pool(name="psS", bufs=1, space="PSUM"))
    psU = ctx.enter_context(tc.tile_pool(name="psU", bufs=1, space="PSUM"))
    psW = ctx.enter_context(tc.tile_pool(name="psW", bufs=1, space="PSUM"))

    # ---- persistent SBUF + ordered input DMAs ----------------------------
    xT_sb = [wpool.tile([128, NT], BF, tag=f"xT{i}", name=f"xT{i}") for i in range(8)]
    wqk_sb = [wpool.tile([128, 1024], BF, tag=f"wqk{i}", name=f"wqk{i}") for i in range(8)]
    wv_sb = [wpool.tile([128, 512], BF, tag=f"wv{i}", name=f"wv{i}") for i in range(8)]
    wp_sb = [wpool.tile([128, 1024], BF, tag=f"wp{i}", name=f"wp{i}") for i in range(4)]
    # earliest needs first: x tokens 0:1024, pair-0 qk weights, wv
    for ts in range(2):
        for i in range(8):
            nc.sync.dma_start(out=xT_sb[i][:, ts * 512:(ts + 1) * 512],
                              in_=xT[i * 128:(i + 1) * 128, ts * 512:(ts + 1) * 512])
    for i in range(8):
        nc.sync.dma_start(out=wqk_sb[i][:, 0:256], in_=wqk[i * 128:(i + 1) * 128, 0:256])
    for i in range(8):
        nc.sync.dma_start(out=wv_sb[i], in_=wv[i * 128:(i + 1) * 128, :])
    for ts in range(2, 4):
        for i in range(8):
            nc.sync.dma_start(out=xT_sb[i][:, ts * 512:(ts + 1) * 512],
                              in_=xT[i * 128:(i + 1) * 128, ts * 512:(ts + 1) * 512])
    for p in range(1, 4):
        for i in range(8):
            nc.sync.dma_start(out=wqk_sb[i][:, p * 256:(p + 1) * 256],
                              in_=wqk[i * 128:(i + 1) * 128, p * 256:(p + 1) * 256])
    for i in range(4):
        nc.sync.dma_start(out=wp_sb[i], in_=wp[i * 128:(i + 1) * 128, :])

    # qkT[2p] = Q features of pair p (rows: headA 0:64 | headB 64:128 hd),
    # qkT[2p+1] = K features of pair p.  cols = 2048 tokens.
    qkT = [wpool.tile([128, NT], BF, tag=f"qkT{f}", name=f"qkT{f}") for f in range(8)]
    v_sb = [wpool.tile([128, 512], BF, tag=f"v{t}", name=f"v{t}") for t in range(16)]
    uhat = [wpool.tile([128, NT], BF, tag=f"uh{p}", name=f"uh{p}") for p in range(4)]
    ones64 = wpool.tile([128, 64], BF, tag="ones64", name="ones64")
    nc.vector.memset(ones64, 1.0)

    # three independent 2-bank score tensors rotating per kt: exp-op j reads
    # one whole tile; reuse distance 1.5 kt keeps next-kt chunks WAR-free
    stP = [psS.tile([128, 1024], F32, tag=f"stP{j}", name=f"stP{j}") for j in range(3)]

    # ---- dense chains (M=128, (128,128) mode) ----------------------------
    def qk_chain(f, ts2):
        scr = psW.tile([128, 512], F32, tag="scr", name=f"qk{f}_{ts2}")
        for d in range(8):
            nc.tensor.matmul(scr, wqk_sb[d][:, f * 128:(f + 1) * 128],
                             xT_sb[d][:, ts2 * 512:(ts2 + 1) * 512],
                             start=(d == 0), stop=(d == 7))
        nc.vector.tensor_copy(out=qkT[f][:, ts2 * 512:(ts2 + 1) * 512], in_=scr)

    def v_chain(t):
        scr = psW.tile([128, 512], F32, tag="scr", name=f"v{t}")
        for d in range(8):
            nc.tensor.matmul(scr, xT_sb[d][:, t * 128:(t + 1) * 128], wv_sb[d],
                             start=(d == 0), stop=(d == 7))
        nc.vector.tensor_copy(out=v_sb[t], in_=scr)
        v_emitted[t] = True

    def proj_chain(qt, es):
        scr = psW.tile([128, 512], F32, tag="scr", name=f"pj{qt}_{es}")
        for c in range(4):
            nc.tensor.matmul(scr, uhat[c][:, qt * 128:(qt + 1) * 128],
                             wp_sb[c][:, es * 512:(es + 1) * 512],
                             start=(c == 0), stop=(c == 3))
        ot = opool.tile([128, 512], F32, tag="out", name=f"ot{qt}_{es}")
        nc.vector.tensor_copy(out=ot, in_=scr)
        nc.sync.dma_start(out=y[qt * 128:(qt + 1) * 128, es * 512:(es + 1) * 512], in_=ot)

    # ---- attention state --------------------------------------------------
    v_emitted = [False] * 16
    gkt = [0]  # global kt counter -> stP rotation
    eS = {}  # (hidx, s, kt) -> exp'd scores [128, 1024] = [A | B]
    eacc = {}  # (hidx, s) -> accumulated E
    recs = {}  # (hidx, s) -> (recA, recB)
    passes = []  # pv pass FIFO: dicts

    def emit_scores_exps(hidx, p, h, kt):
        qk_q, qk_k = qkT[2 * p], qkT[2 * p + 1]
        g = gkt[0]
        gkt[0] += 1
        tiles = (stP[(2 * g) % 3], stP[(2 * g + 1) % 3])
        for s in range(2):  # chunk pair (A, B) for q-slice s
            q0 = h * 1024 + s * 512
            for head in range(2):
                r = slice(head * 64, head * 64 + 64)
                ch = tiles[s][:, head * 512:(head + 1) * 512]
                for lh in range(2):
                    k0 = kt * 128 + lh * 64
                    nc.tensor.matmul(ch[lh * 64:(lh + 1) * 64, :], qk_k[r, k0:k0 + 64],
                                     qk_q[r, q0:q0 + 512], start=True, stop=True)
        for s, pool in ((0, e0pool), (1, e1pool)):
            et = pool.tile([128, 1024], BF, tag=f"e{s}", name=f"e{hidx}_{s}_{kt}")
            nc.scalar.activation(out=et, in_=tiles[s][:, :], func=EXP, scale=SCALE)
            eS[(hidx, s, kt)] = et
            if kt == 0:
                ea = eapool.tile([128, 1024], BF, tag=f"ea{s}", name=f"ea{hidx}_{s}")
                eacc[(hidx, s)] = ea
                nc.vector.tensor_copy(out=ea, in_=et)
            else:
                nc.vector.tensor_add(out=eacc[(hidx, s)], in0=eacc[(hidx, s)], in1=et)

    def rec_chain(hidx, s):
        ea = eacc[(hidx, s)]
        sp = psW.tile([128, 512], F32, tag="scr", name=f"sums{hidx}_{s}")
        nc.tensor.matmul(sp[0:64, :], ones64, ea[:, 0:512], start=True, stop=True)
        nc.tensor.matmul(sp[64:128, :], ones64, ea[:, 512:1024], start=True, stop=True)
        ss = spool.tile([128, 512], F32, tag="sums", name=f"ss{hidx}_{s}")
        nc.vector.tensor_copy(out=ss, in_=sp)
        out = []
        for hb in (0, 1):
            rsp = spool.tile([128, 4], F32, tag="rsp", name=f"rsp{hidx}_{s}_{hb}")
            row = ss[hb * 64:hb * 64 + 1, :].rearrange("p (a b) -> p a b", a=128)
            nc.gpsimd.dma_start(out=rsp, in_=row)
            rspr = spool.tile([128, 4], BF, tag="rspr", name=f"rspr{hidx}_{s}_{hb}")
            with nc.allow_low_precision(reason="bf16 softmax denominators are within tolerance"):
                nc.vector.reciprocal(out=rspr, in_=rsp)
            rrow = spool.tile([1, 512], BF, tag="rrow", bufs=1, name=f"rrow{hidx}_{s}_{hb}")
            nc.gpsimd.dma_start(out=rrow[0:1, :].rearrange("p (a b) -> p a b", a=128), in_=rspr)
            rec = recpool.tile([128, 512], BF, tag="rec", name=f"rec{hidx}_{s}_{hb}")
            nc.gpsimd.partition_broadcast(out_ap=rec[:, :], in_ap=rrow[0:1, :])
            out.append(rec)
        recs[(hidx, s)] = out

    def emit_norm(ps):
        # drain ut immediately (frees the single PSUM bank), then normalize
        # in place in SBUF once the reciprocal broadcast lands.
        p, h, s, hidx = ps["p"], ps["h"], ps["s"], ps["hidx"]
        recA, recB = recs[(hidx, s)]
        ucols = slice(h * 1024 + s * 512, h * 1024 + (s + 1) * 512)
        nc.vector.tensor_copy(out=uhat[p][:, ucols], in_=ps["ut"])
        nc.vector.tensor_mul(uhat[p][0:64, ucols], uhat[p][0:64, ucols], recA[0:64, :])
        nc.vector.tensor_mul(uhat[p][64:128, ucols], uhat[p][64:128, ucols], recB[64:128, :])

    def pump_pv(cur_hidx, cur_step, budget):
        while budget > 0 and passes:
            ps = passes[0]
            if ps["kt"] >= 16:
                if recs.get((ps["hidx"], ps["s"])) is None:
                    break  # rec not emitted yet (same-half): wait for end block
                emit_norm(ps)
                passes.pop(0)
                continue
            kt = ps["kt"]
            if ps["hidx"] == cur_hidx and kt > cur_step - 2:
                break
            if not v_emitted[kt]:
                break
            if ps["ut"] is None:
                ps["ut"] = psU.tile([128, 512], F32, tag="ut", name=f"ut{ps['hidx']}_{ps['s']}")
            e = eS.pop((ps["hidx"], ps["s"], kt))
            p = ps["p"]
            st, sp = (kt == 0), (kt == 15)
            nc.tensor.matmul(ps["ut"][0:64, :], v_sb[kt][:, p * 128:p * 128 + 64], e[:, 0:512], start=st, stop=sp)
            nc.tensor.matmul(ps["ut"][64:128, :], v_sb[kt][:, p * 128 + 64:(p + 1) * 128], e[:, 512:1024], start=st, stop=sp)
            ps["kt"] += 1
            budget -= 1

    # ---- dense filler schedule -------------------------------------------
    QK = lambda f, t: (lambda: qk_chain(f, t))
    V = lambda t: (lambda: v_chain(t))
    PJ = lambda qt, es: (lambda: proj_chain(qt, es))
    fillers = [
        # p0h0: v stream + K-p0 incremental + pair-1 features early
        [V(2), QK(1, 1), V(3), QK(2, 0), V(4), QK(1, 2), V(5), QK(2, 1),
         V(6), QK(1, 3), V(7), QK(3, 0), V(8), V(9), V(10), V(11)],
        # p1h0
        [V(12), V(13), QK(3, 1), V(14), QK(3, 2), V(15), QK(3, 3), QK(4, 0),
         QK(4, 1), QK(5, 0), QK(5, 1)],
        # p2h0
        [QK(5, 2), QK(5, 3), QK(6, 0), QK(6, 1), QK(7, 0), QK(7, 1)],
        # p3h0
        [QK(7, 2), QK(7, 3), QK(0, 2), QK(0, 3), QK(2, 2), QK(2, 3)],
        # p0h1
        [QK(4, 2), QK(4, 3), QK(6, 2), QK(6, 3)],
        # p1h1 .. p3h1: proj for h0 q-tiles
        [PJ(qt, es) for qt in range(0, 3) for es in range(2)],
        [PJ(qt, es) for qt in range(3, 6) for es in range(2)],
        [PJ(qt, es) for qt in range(6, 8) for es in range(2)],
    ]

    # ---- lead-in ----------------------------------------------------------
    qk_chain(0, 0)
    qk_chain(0, 1)
    qk_chain(1, 0)
    v_chain(0)
    v_chain(1)

    # ---- main loop --------------------------------------------------------
    for hidx, (p, h) in enumerate(HALVES):
        fl = list(fillers[hidx])
        for kt in range(16):
            if fl:
                fl.pop(0)()
            pump_pv(hidx, kt, 3)
            emit_scores_exps(hidx, p, h, kt)
            if kt == 8:
                passes.append({"hidx": hidx, "p": p, "h": h, "s": 0, "kt": 0, "ut": None})
            pump_pv(hidx, kt, 3)
        rec_chain(hidx, 0)
        rec_chain(hidx, 1)
        passes.append({"hidx": hidx, "p": p, "h": h, "s": 1, "kt": 0, "ut": None})

    # ---- tail -------------------------------------------------------------
    guard = 0
    while passes and guard < 200:
        pump_pv(-1, 99, 4)
        guard += 1
    for qt in range(8, 16):
        for es in range(2):
            proj_chain(qt, es)


_NC_CACHE = {}


def _build_nc():
    if "nc" in _NC_CACHE:
        return _NC_CACHE["nc"]
    nc = bacc.Bacc("TRN2", target_bir_lowering=False, debug=False, num_devices=N_CORES)
    xT = nc.dram_tensor("xT", [D, NT], BF, kind="ExternalInput").ap()
    wqk = nc.dram_tensor("wqk", [D, 1024], BF, kind="ExternalInput").ap()
    wv = nc.dram_tensor("wv", [D, 512], BF, kind="ExternalInput").ap()
    wp = nc.dram_tensor("wp", [512, 1024], BF, kind="ExternalInput").ap()
    y = nc.dram_tensor("y", [NT, 1024], F32, kind="ExternalOutput").ap()
    from contextlib import ExitStack

    with tile.TileContext(nc) as tc, ExitStack() as ctx:
        _body(tc, ctx, y, xT, wqk, wv, wp)
    nc.compile()
    _NC_CACHE["nc"] = nc
    return nc


def _prepare_in_maps(x, W_qkv, W_proj):
    x = np.asarray(x, dtype=np.float32)
    W_qkv = np.asarray(W_qkv, dtype=np.float32)
    W_proj = np.asarray(W_proj, dtype=np.float32)
    in_maps = []
    for c in range(N_CORES):
        b, hg = divmod(c, 2)
        cs = slice(hg * 512, (hg + 1) * 512)
        xTc = np.ascontiguousarray(x[b].T).astype(BF16)
        Qc = W_qkv[:, 0:1024][:, cs]
        Kc = W_qkv[:, 1024:2048][:, cs]
        # per-pair interleave: [Q-pair0 | K-pair0 | Q-pair1 | K-pair1 | ...]
        blocks = []
        for p in range(4):
            blocks.append(Qc[:, p * 128:(p + 1) * 128])
            blocks.append(Kc[:, p * 128:(p + 1) * 128])
        wqk = np.ascontiguousarray(np.concatenate(blocks, axis=1)).astype(BF16)
        wv = np.ascontiguousarray(W_qkv[:, 2048:3072][:, cs]).astype(BF16)
        wp = np.ascontiguousarray(W_proj[cs, :]).astype(BF16)
        in_maps.append({"xT": xTc, "wqk": wqk, "wv": wv, "wp": wp})
    return in_maps


def _run(x, W_qkv, W_proj, b_proj, trace=False):
    nc = _build_nc()
    in_maps = _prepare_in_maps(x, W_qkv, W_proj)
    res = bass_utils.run_bass_kernel_spmd(
        nc, in_maps, core_ids=list(range(N_CORES)), trace=trace
    )
    b_proj = np.asarray(b_proj, dtype=np.float32)
    y = np.empty((4, NT, D), dtype=np.float32)
    for b in range(4):
        y[b] = res.results[2 * b]["y"] + res.results[2 * b + 1]["y"] + b_proj[None, :]
    return y, res


def kernel(x, W_qkv, W_proj, b_proj):
    y, _ = _run(x, W_qkv, W_proj, b_proj, trace=False)
    return y


# revision 15
# speedup vs baseline: 1.5028x; 1.0006x over previous
"""Trainium2 Bass kernel for nn_Attention (B=4, N=2048, D=1024, H=16, Hd=64).

Sharding: 8 cores = 4 batches x 2 head-groups; core c: batch c//2, heads
[(c%2)*8, (c%2)*8+8). Host sums the two partial projections per batch + bias.

v2 design (vs v1): keeps ScalarE (the 257us exp floor) saturated and pushes
TensorE below it via PE array tiling:
  - scores run as 4-way-concurrent 64x64 array tiles (2 heads x keys-lo/hi),
    ~216ns per [128 keys x 2x512 q] group (measured), using a 6-slice PSUM
    rotation [128,3072] so consecutive steps' chunks never WAR-block.
  - exp ops are [128,1024] PSUM->SBUF (1005ns measured) and always have
    inputs ready >=2 ops ahead (slice reuse distance 1.5 kt).
  - qkv/proj dense chains + pv (attn@V) all run as (128,64) column-paired
    MMs so per-step the PE sees only two tiling-mode switches.
  - softmax denominators: eacc adds on DVE (+ every 4th on GpSimd),
    ones-matmul partition reduce, reciprocal + gpsimd broadcast, with the
    normalize fused into the PSUM->SBUF drain (tensor_mul from PSUM).
  - U^T accumulates in a single PSUM bank [128,512] per (half, q-slice);
    the two q-slice passes per half are pipelined across half boundaries.
"""

import os
import sys
import types

import numpy as np

for _p in ("/opt/trn_rl_repo", "/root/.axon_site/_ro/trn_rl_repo"):
    if _p not in sys.path and os.path.isdir(_p):
        sys.path.append(_p)

import ml_dtypes  # noqa: E402

BF16 = ml_dtypes.bfloat16


def _install_ntff_shim():
    if "antenv.axon_hooks" in sys.modules:
        return
    mod = types.ModuleType("antenv.axon_hooks")
    mod._hook = None
    mod.set_axon_ntff_profile_hook = lambda h: setattr(mod, "_hook", h)
    mod.get_axon_ntff_profile_hook = lambda: mod._hook
    sys.modules["antenv.axon_hooks"] = mod
    try:
        import antenv

        antenv.axon_hooks = mod
    except ImportError:
        pass
    try:
        from trn_agent_boot.trn_boot import _ntff_profile_via_ctypes

        hook = _ntff_profile_via_ctypes("/opt/axon/libaxon_pjrt.so")
        if hook is not None:
            mod.set_axon_ntff_profile_hook(hook)
    except Exception:
        pass


_install_ntff_shim()

import concourse.bacc as bacc  # noqa: E402
import concourse.tile as tile  # noqa: E402
from concourse import mybir  # noqa: E402
import concourse.bass_utils as bass_utils  # noqa: E402

bass_utils.upload_artifacts = lambda tmpdir: tmpdir

F32 = mybir.dt.float32
BF = mybir.dt.bfloat16
EXP = mybir.ActivationFunctionType.Exp

N_CORES = 8
NT = 2048
D = 1024
HD = 64
SCALE = HD**-0.5

HALVES = [(0, 0), (1, 0), (2, 0), (3, 0), (0, 1), (1, 1), (2, 1), (3, 1)]


def _body(tc: "tile.TileContext", ctx, y, xT, wqk, wv, wp):
    nc = tc.nc

    wpool = ctx.enter_context(tc.tile_pool(name="wpool", bufs=1))
    e0pool = ctx.enter_context(tc.tile_pool(name="e0pool", bufs=8))
    e1pool = ctx.enter_context(tc.tile_pool(name="e1pool", bufs=17))
    eapool = ctx.enter_context(tc.tile_pool(name="eapool", bufs=2))
    spool = ctx.enter_context(tc.tile_pool(name="spool", bufs=2))
    recpool = ctx.enter_context(tc.tile_pool(name="recpool", bufs=4))
    opool = ctx.enter_context(tc.tile_pool(name="opool", bufs=2))
    psS = ctx.enter_context(tc.tile_pool(name="psS", bufs=1, space="PSUM"))
    psU = ctx.enter_context(tc.tile_pool(name="psU", bufs=1, space="PSUM"))
    psW = ctx.enter_context(tc.tile_pool(name="psW", bufs=1, space="PSUM"))

    # ---- persistent SBUF + ordered input DMAs ----------------------------
    xT_sb = [wpool.tile([128, NT], BF, tag=f"xT{i}", name=f"xT{i}") for i in range(8)]
    wqk_sb = [wpool.tile([128, 1024], BF, tag=f"wqk{i}", name=f"wqk{i}") for i in range(8)]
    wv_sb = [wpool.tile([128, 512], BF, tag=f"wv{i}", name=f"wv{i}") for i in range(8)]
    wp_sb = [wpool.tile([128, 1024], BF, tag=f"wp{i}", name=f"wp{i}") for i in range(4)]
    # earliest needs first: x tokens 0:1024, pair-0 qk weights, wv
    for ts in range(2):
        for i in range(8):
            nc.sync.dma_start(out=xT_sb[i][:, ts * 512:(ts + 1) * 512],
                              in_=xT[i * 128:(i + 1) * 128, ts * 512:(ts + 1) * 512])
    for i in range(8):
        nc.sync.dma_start(out=wqk_sb[i][:, 0:256], in_=wqk[i * 128:(i + 1) * 128, 0:256])
    for i in range(8):
        nc.sync.dma_start(out=wv_sb[i], in_=wv[i * 128:(i + 1) * 128, :])
    for ts in range(2, 4):
        for i in range(8):
            nc.sync.dma_start(out=xT_sb[i][:, ts * 512:(ts + 1) * 512],
                              in_=xT[i * 128:(i + 1) * 128, ts * 512:(ts + 1) * 512])
    for p in range(1, 4):
        for i in range(8):
            nc.sync.dma_start(out=wqk_sb[i][:, p * 256:(p + 1) * 256],
                              in_=wqk[i * 128:(i + 1) * 128, p * 256:(p + 1) * 256])
    for i in range(4):
        nc.sync.dma_start(out=wp_sb[i], in_=wp[i * 128:(i + 1) * 128, :])

    # qkT[2p] = Q features of pair p (rows: headA 0:64 | headB 64:128 hd),
    # qkT[2p+1] = K features of pair p.  cols = 2048 tokens.
    qkT = [wpool.tile([128, NT], BF, tag=f"qkT{f}", name=f"qkT{f}") for f in range(8)]
    v_sb = [wpool.tile([128, 512], BF, tag=f"v{t}", name=f"v{t}") for t in range(16)]
    uhat = [wpool.tile([128, NT], BF, tag=f"uh{p}", name=f"uh{p}") for p in range(4)]
    ones64 = wpool.tile([128, 64], BF, tag="ones64", name="ones64")
    nc.vector.memset(ones64, 1.0)

    # three independent 2-bank score tensors rotating per kt: exp-op j reads
    # one whole tile; reuse distance 1.5 kt keeps next-kt chunks WAR-free
    stP = [psS.tile([128, 1024], F32, tag=f"stP{j}", name=f"stP{j}") for j in range(3)]

    # ---- dense chains (M=128, (128,128) mode) ----------------------------
    def qk_chain(f, ts2, alt=False):
        pool, tg = (psU, "ut") if alt else (psW, "scr")
        scr = pool.tile([128, 512], F32, tag=tg, name=f"qk{f}_{ts2}")
        for d in range(8):
            nc.tensor.matmul(scr, wqk_sb[d][:, f * 128:(f + 1) * 128],
                             xT_sb[d][:, ts2 * 512:(ts2 + 1) * 512],
                             start=(d == 0), stop=(d == 7))
        nc.vector.tensor_copy(out=qkT[f][:, ts2 * 512:(ts2 + 1) * 512], in_=scr)

    def v_chain(t, alt=False):
        pool, tg = (psU, "ut") if alt else (psW, "scr")
        scr = pool.tile([128, 512], F32, tag=tg, name=f"v{t}")
        for d in range(8):
            nc.tensor.matmul(scr, xT_sb[d][:, t * 128:(t + 1) * 128], wv_sb[d],
                             start=(d == 0), stop=(d == 7))
        nc.vector.tensor_copy(out=v_sb[t], in_=scr)
        v_emitted[t] = True

    def proj_chain(qt, es):
        scr = psW.tile([128, 512], F32, tag="scr", name=f"pj{qt}_{es}")
        for c in range(4):
            nc.tensor.matmul(scr, uhat[c][:, qt * 128:(qt + 1) * 128],
                             wp_sb[c][:, es * 512:(es + 1) * 512],
                             start=(c == 0), stop=(c == 3))
        ot = opool.tile([128, 512], F32, tag="out", name=f"ot{qt}_{es}")
        nc.vector.tensor_copy(out=ot, in_=scr)
        nc.sync.dma_start(out=y[qt * 128:(qt + 1) * 128, es * 512:(es + 1) * 512], in_=ot)

    # ---- attention state --------------------------------------------------
    v_emitted = [False] * 16
    gkt = [0]  # global kt counter -> stP rotation
    eS = {}  # (hidx, s, kt) -> exp'd scores [128, 1024] = [A | B]
    eacc = {}  # (hidx, s) -> accumulated E
    recs = {}  # (hidx, s) -> (recA, recB)
    passes = []  # pv pass FIFO: dicts

    def emit_scores_exps(hidx, p, h, kt):
        qk_q, qk_k = qkT[2 * p], qkT[2 * p + 1]
        g = gkt[0]
        gkt[0] += 1
        tiles = (stP[(2 * g) % 3], stP[(2 * g + 1) % 3])
        for s in range(2):  # chunk pair (A, B) for q-slice s
            q0 = h * 1024 + s * 512
            for head in range(2):
                r = slice(head * 64, head * 64 + 64)
                ch = tiles[s][:, head * 512:(head + 1) * 512]
                for lh in range(2):
                    k0 = kt * 128 + lh * 64
                    nc.tensor.matmul(ch[lh * 64:(lh + 1) * 64, :], qk_k[r, k0:k0 + 64],
                                     qk_q[r, q0:q0 + 512], start=True, stop=True)
        for s, pool in ((0, e0pool), (1, e1pool)):
            et = pool.tile([128, 1024], BF, tag=f"e{s}", name=f"e{hidx}_{s}_{kt}")
            nc.scalar.activation(out=et, in_=tiles[s][:, :], func=EXP, scale=SCALE)
            eS[(hidx, s, kt)] = et
            if kt == 0:
                ea = eapool.tile([128, 1024], BF, tag=f"ea{s}", name=f"ea{hidx}_{s}")
                eacc[(hidx, s)] = ea
                nc.vector.tensor_copy(out=ea, in_=et)
            else:
                nc.vector.tensor_add(out=eacc[(hidx, s)], in0=eacc[(hidx, s)], in1=et)

    def rec_chain(hidx, s):
        ea = eacc[(hidx, s)]
        sp = psW.tile([128, 512], F32, tag="scr", name=f"sums{hidx}_{s}")
        nc.tensor.matmul(sp[0:64, :], ones64, ea[:, 0:512], start=True, stop=True)
        nc.tensor.matmul(sp[64:128, :], ones64, ea[:, 512:1024], start=True, stop=True)
        ss = spool.tile([128, 512], F32, tag="sums", name=f"ss{hidx}_{s}")
        nc.vector.tensor_copy(out=ss, in_=sp)
        out = []
        for hb in (0, 1):
            rsp = spool.tile([128, 4], F32, tag="rsp", name=f"rsp{hidx}_{s}_{hb}")
            row = ss[hb * 64:hb * 64 + 1, :].rearrange("p (a b) -> p a b", a=128)
            nc.gpsimd.dma_start(out=rsp, in_=row)
            rspr = spool.tile([128, 4], BF, tag="rspr", name=f"rspr{hidx}_{s}_{hb}")
            with nc.allow_low_precision(reason="bf16 softmax denominators are within tolerance"):
                nc.vector.reciprocal(out=rspr, in_=rsp)
            rrow = spool.tile([1, 512], BF, tag="rrow", bufs=1, name=f"rrow{hidx}_{s}_{hb}")
            nc.gpsimd.dma_start(out=rrow[0:1, :].rearrange("p (a b) -> p a b", a=128), in_=rspr)
            rec = recpool.tile([128, 512], BF, tag="rec", name=f"rec{hidx}_{s}_{hb}")
            nc.gpsimd.partition_broadcast(out_ap=rec[:, :], in_ap=rrow[0:1, :])
            out.append(rec)
        recs[(hidx, s)] = out

    def emit_norm(ps):
        # drain ut immediately (frees the single PSUM bank), then normalize
        # in place in SBUF once the reciprocal broadcast lands.
        p, h, s, hidx = ps["p"], ps["h"], ps["s"], ps["hidx"]
        recA, recB = recs[(hidx, s)]
        ucols = slice(h * 1024 + s * 512, h * 1024 + (s + 1) * 512)
        nc.vector.tensor_copy(out=uhat[p][:, ucols], in_=ps["ut"])
        nc.vector.tensor_mul(uhat[p][0:64, ucols], uhat[p][0:64, ucols], recA[0:64, :])
        nc.vector.tensor_mul(uhat[p][64:128, ucols], uhat[p][64:128, ucols], recB[64:128, :])

    def pump_pv(cur_hidx, cur_step, budget):
        while budget > 0 and passes:
            ps = passes[0]
            if ps["kt"] >= 16:
                if recs.get((ps["hidx"], ps["s"])) is None:
                    break  # rec not emitted yet (same-half): wait for end block
                emit_norm(ps)
                passes.pop(0)
                continue
            kt = ps["kt"]
            if ps["hidx"] == cur_hidx and kt > cur_step - 2:
                break
            if not v_emitted[kt]:
                break
            if ps["ut"] is None:
                ps["ut"] = psU.tile([128, 512], F32, tag="ut", name=f"ut{ps['hidx']}_{ps['s']}")
            e = eS.pop((ps["hidx"], ps["s"], kt))
            p = ps["p"]
            st, sp = (kt == 0), (kt == 15)
            nc.tensor.matmul(ps["ut"][0:64, :], v_sb[kt][:, p * 128:p * 128 + 64], e[:, 0:512], start=st, stop=sp)
            nc.tensor.matmul(ps["ut"][64:128, :], v_sb[kt][:, p * 128 + 64:(p + 1) * 128], e[:, 512:1024], start=st, stop=sp)
            ps["kt"] += 1
            budget -= 1

    # ---- dense filler schedule -------------------------------------------
    QK = lambda f, t, a=False: (lambda: qk_chain(f, t, a))
    V = lambda t, a=False: (lambda: v_chain(t, a))
    PJ = lambda qt, es: (lambda: proj_chain(qt, es))
    fillers = [
        # p0h0: v stream + K-p0 incremental + pair-1 features early; the
        # first chains double-buffer through the still-idle ut bank
        [V(2, True), QK(1, 1), V(3, True), QK(2, 0), V(4, True), QK(1, 2),
         V(5, True), QK(2, 1), V(6), QK(1, 3), V(7), QK(3, 0), V(8), V(9),
         V(10), V(11)],
        # p1h0
        [V(12), V(13), QK(3, 1), V(14), QK(3, 2), V(15), QK(3, 3), QK(4, 0),
         QK(4, 1), QK(5, 0), QK(5, 1)],
        # p2h0
        [QK(5, 2), QK(5, 3), QK(6, 0), QK(6, 1), QK(7, 0), QK(7, 1)],
        # p3h0
        [QK(7, 2), QK(7, 3), QK(0, 2), QK(0, 3), QK(2, 2), QK(2, 3)],
        # p0h1
        [QK(4, 2), QK(4, 3), QK(6, 2), QK(6, 3)],
        # p1h1 .. p3h1: proj for h0 q-tiles
        [PJ(qt, es) for qt in range(0, 3) for es in range(2)],
        [PJ(qt, es) for qt in range(3, 6) for es in range(2)],
        [PJ(qt, es) for qt in range(6, 8) for es in range(2)],
    ]

    # ---- lead-in ----------------------------------------------------------
    qk_chain(0, 0)
    qk_chain(0, 1, alt=True)
    qk_chain(1, 0)
    v_chain(0, alt=True)
    v_chain(1)

    # ---- main loop --------------------------------------------------------
    for hidx, (p, h) in enumerate(HALVES):
        fl = list(fillers[hidx])
        for kt in range(16):
            if fl:
                fl.pop(0)()
            pump_pv(hidx, kt, 6 if kt < 4 else 3)
            emit_scores_exps(hidx, p, h, kt)
            if kt == 8:
                passes.append({"hidx": hidx, "p": p, "h": h, "s": 0, "kt": 0, "ut": None})
            pump_pv(hidx, kt, 3)
        rec_chain(hidx, 0)
        rec_chain(hidx, 1)
        passes.append({"hidx": hidx, "p": p, "h": h, "s": 1, "kt": 0, "ut": None})

    # ---- tail -------------------------------------------------------------
    guard = 0
    while passes and guard < 200:
        pump_pv(-1, 99, 4)
        guard += 1
    for qt in range(8, 16):
        for es in range(2):
            proj_chain(qt, es)


_NC_CACHE = {}


def _build_nc():
    if "nc" in _NC_CACHE:
        return _NC_CACHE["nc"]
    nc = bacc.Bacc("TRN2", target_bir_lowering=False, debug=False, num_devices=N_CORES)
    xT = nc.dram_tensor("xT", [D, NT], BF, kind="ExternalInput").ap()
    wqk = nc.dram_tensor("wqk", [D, 1024], BF, kind="ExternalInput").ap()
    wv = nc.dram_tensor("wv", [D, 512], BF, kind="ExternalInput").ap()
    wp = nc.dram_tensor("wp", [512, 1024], BF, kind="ExternalInput").ap()
    y = nc.dram_tensor("y", [NT, 1024], F32, kind="ExternalOutput").ap()
    from contextlib import ExitStack

    with tile.TileContext(nc) as tc, ExitStack() as ctx:
        _body(tc, ctx, y, xT, wqk, wv, wp)
    nc.compile()
    _NC_CACHE["nc"] = nc
    return nc


def _prepare_in_maps(x, W_qkv, W_proj):
    x = np.asarray(x, dtype=np.float32)
    W_qkv = np.asarray(W_qkv, dtype=np.float32)
    W_proj = np.asarray(W_proj, dtype=np.float32)
    in_maps = []
    for c in range(N_CORES):
        b, hg = divmod(c, 2)
        cs = slice(hg * 512, (hg + 1) * 512)
        xTc = np.ascontiguousarray(x[b].T).astype(BF16)
        Qc = W_qkv[:, 0:1024][:, cs]
        Kc = W_qkv[:, 1024:2048][:, cs]
        # per-pair interleave: [Q-pair0 | K-pair0 | Q-pair1 | K-pair1 | ...]
        blocks = []
        for p in range(4):
            blocks.append(Qc[:, p * 128:(p + 1) * 128])
            blocks.append(Kc[:, p * 128:(p + 1) * 128])
        wqk = np.ascontiguousarray(np.concatenate(blocks, axis=1)).astype(BF16)
        wv = np.ascontiguousarray(W_qkv[:, 2048:3072][:, cs]).astype(BF16)
        wp = np.ascontiguousarray(W_proj[cs, :]).astype(BF16)
        in_maps.append({"xT": xTc, "wqk": wqk, "wv": wv, "wp": wp})
    return in_maps


def _run(x, W_qkv, W_proj, b_proj, trace=False):
    nc = _build_nc()
    in_maps = _prepare_in_maps(x, W_qkv, W_proj)
    res = bass_utils.run_bass_kernel_spmd(
        nc, in_maps, core_ids=list(range(N_CORES)), trace=trace
    )
    b_proj = np.asarray(b_proj, dtype=np.float32)
    y = np.empty((4, NT, D), dtype=np.float32)
    for b in range(4):
        y[b] = res.results[2 * b]["y"] + res.results[2 * b + 1]["y"] + b_proj[None, :]
    return y, res


def kernel(x, W_qkv, W_proj, b_proj):
    y, _ = _run(x, W_qkv, W_proj, b_proj, trace=False)
    return y


# revision 16
# speedup vs baseline: 1.5125x; 1.0065x over previous
"""Trainium2 Bass kernel for nn_Attention (B=4, N=2048, D=1024, H=16, Hd=64).

Sharding: 8 cores = 4 batches x 2 head-groups; core c: batch c//2, heads
[(c%2)*8, (c%2)*8+8). Host sums the two partial projections per batch + bias.

v2 design (vs v1): keeps ScalarE (the 257us exp floor) saturated and pushes
TensorE below it via PE array tiling:
  - scores run as 4-way-concurrent 64x64 array tiles (2 heads x keys-lo/hi),
    ~216ns per [128 keys x 2x512 q] group (measured), using a 6-slice PSUM
    rotation [128,3072] so consecutive steps' chunks never WAR-block.
  - exp ops are [128,1024] PSUM->SBUF (1005ns measured) and always have
    inputs ready >=2 ops ahead (slice reuse distance 1.5 kt).
  - qkv/proj dense chains + pv (attn@V) all run as (128,64) column-paired
    MMs so per-step the PE sees only two tiling-mode switches.
  - softmax denominators: eacc adds on DVE (+ every 4th on GpSimd),
    ones-matmul partition reduce, reciprocal + gpsimd broadcast, with the
    normalize fused into the PSUM->SBUF drain (tensor_mul from PSUM).
  - U^T accumulates in a single PSUM bank [128,512] per (half, q-slice);
    the two q-slice passes per half are pipelined across half boundaries.
"""

import os
import sys
import types

import numpy as np

for _p in ("/opt/trn_rl_repo", "/root/.axon_site/_ro/trn_rl_repo"):
    if _p not in sys.path and os.path.isdir(_p):
        sys.path.append(_p)

import ml_dtypes  # noqa: E402

BF16 = ml_dtypes.bfloat16


def _install_ntff_shim():
    if "antenv.axon_hooks" in sys.modules:
        return
    mod = types.ModuleType("antenv.axon_hooks")
    mod._hook = None
    mod.set_axon_ntff_profile_hook = lambda h: setattr(mod, "_hook", h)
    mod.get_axon_ntff_profile_hook = lambda: mod._hook
    sys.modules["antenv.axon_hooks"] = mod
    try:
        import antenv

        antenv.axon_hooks = mod
    except ImportError:
        pass
    try:
        from trn_agent_boot.trn_boot import _ntff_profile_via_ctypes

        hook = _ntff_profile_via_ctypes("/opt/axon/libaxon_pjrt.so")
        if hook is not None:
            mod.set_axon_ntff_profile_hook(hook)
    except Exception:
        pass


_install_ntff_shim()

import concourse.bacc as bacc  # noqa: E402
import concourse.tile as tile  # noqa: E402
from concourse import mybir  # noqa: E402
import concourse.bass_utils as bass_utils  # noqa: E402

bass_utils.upload_artifacts = lambda tmpdir: tmpdir

F32 = mybir.dt.float32
BF = mybir.dt.bfloat16
EXP = mybir.ActivationFunctionType.Exp

N_CORES = 8
NT = 2048
D = 1024
HD = 64
SCALE = HD**-0.5

HALVES = [(0, 0), (1, 0), (2, 0), (3, 0), (0, 1), (1, 1), (2, 1), (3, 1)]


def _body(tc: "tile.TileContext", ctx, y, xT, wqk, wv, wp):
    nc = tc.nc

    wpool = ctx.enter_context(tc.tile_pool(name="wpool", bufs=1))
    e0pool = ctx.enter_context(tc.tile_pool(name="e0pool", bufs=8))
    e1pool = ctx.enter_context(tc.tile_pool(name="e1pool", bufs=17))
    eapool = ctx.enter_context(tc.tile_pool(name="eapool", bufs=2))
    spool = ctx.enter_context(tc.tile_pool(name="spool", bufs=2))
    recpool = ctx.enter_context(tc.tile_pool(name="recpool", bufs=4))
    opool = ctx.enter_context(tc.tile_pool(name="opool", bufs=2))
    psS = ctx.enter_context(tc.tile_pool(name="psS", bufs=1, space="PSUM"))
    psU = ctx.enter_context(tc.tile_pool(name="psU", bufs=1, space="PSUM"))
    psW = ctx.enter_context(tc.tile_pool(name="psW", bufs=1, space="PSUM"))

    # ---- persistent SBUF + ordered input DMAs ----------------------------
    xT_sb = [wpool.tile([128, NT], BF, tag=f"xT{i}", name=f"xT{i}") for i in range(8)]
    wqk_sb = [wpool.tile([128, 1024], BF, tag=f"wqk{i}", name=f"wqk{i}") for i in range(8)]
    wv_sb = [wpool.tile([128, 512], BF, tag=f"wv{i}", name=f"wv{i}") for i in range(8)]
    wp_sb = [wpool.tile([128, 1024], BF, tag=f"wp{i}", name=f"wp{i}") for i in range(4)]
    # earliest needs first: x tokens 0:1024, pair-0 qk weights, wv
    for ts in range(2):
        for i in range(8):
            nc.sync.dma_start(out=xT_sb[i][:, ts * 512:(ts + 1) * 512],
                              in_=xT[i * 128:(i + 1) * 128, ts * 512:(ts + 1) * 512])
    for i in range(8):
        nc.sync.dma_start(out=wqk_sb[i][:, 0:256], in_=wqk[i * 128:(i + 1) * 128, 0:256])
    for i in range(8):
        nc.sync.dma_start(out=wv_sb[i], in_=wv[i * 128:(i + 1) * 128, :])
    for ts in range(2, 4):
        for i in range(8):
            nc.sync.dma_start(out=xT_sb[i][:, ts * 512:(ts + 1) * 512],
                              in_=xT[i * 128:(i + 1) * 128, ts * 512:(ts + 1) * 512])
    for p in range(1, 4):
        for i in range(8):
            nc.sync.dma_start(out=wqk_sb[i][:, p * 256:(p + 1) * 256],
                              in_=wqk[i * 128:(i + 1) * 128, p * 256:(p + 1) * 256])
    for i in range(4):
        nc.sync.dma_start(out=wp_sb[i], in_=wp[i * 128:(i + 1) * 128, :])

    # qkT[2p] = Q features of pair p (rows: headA 0:64 | headB 64:128 hd),
    # qkT[2p+1] = K features of pair p.  cols = 2048 tokens.
    qkT = [wpool.tile([128, NT], BF, tag=f"qkT{f}", name=f"qkT{f}") for f in range(8)]
    v_sb = [wpool.tile([128, 512], BF, tag=f"v{t}", name=f"v{t}") for t in range(16)]
    uhat = [wpool.tile([128, NT], BF, tag=f"uh{p}", name=f"uh{p}") for p in range(4)]
    ones64 = wpool.tile([128, 64], BF, tag="ones64", name="ones64")
    nc.vector.memset(ones64, 1.0)

    # three independent 2-bank score tensors rotating per kt: exp-op j reads
    # one whole tile; reuse distance 1.5 kt keeps next-kt chunks WAR-free
    stP = [psS.tile([128, 1024], F32, tag=f"stP{j}", name=f"stP{j}") for j in range(3)]

    # ---- dense chains (M=128, (128,128) mode) ----------------------------
    def qk_chain(f, ts2, alt=False):
        pool, tg = (psU, "ut") if alt else (psW, "scr")
        scr = pool.tile([128, 512], F32, tag=tg, name=f"qk{f}_{ts2}")
        for d in range(8):
            nc.tensor.matmul(scr, wqk_sb[d][:, f * 128:(f + 1) * 128],
                             xT_sb[d][:, ts2 * 512:(ts2 + 1) * 512],
                             start=(d == 0), stop=(d == 7))
        nc.vector.tensor_copy(out=qkT[f][:, ts2 * 512:(ts2 + 1) * 512], in_=scr)

    def v_chain(t, alt=False):
        pool, tg = (psU, "ut") if alt else (psW, "scr")
        scr = pool.tile([128, 512], F32, tag=tg, name=f"v{t}")
        for d in range(8):
            nc.tensor.matmul(scr, xT_sb[d][:, t * 128:(t + 1) * 128], wv_sb[d],
                             start=(d == 0), stop=(d == 7))
        nc.vector.tensor_copy(out=v_sb[t], in_=scr)
        v_emitted[t] = True

    def proj_chain(qt, es):
        scr = psW.tile([128, 512], F32, tag="scr", name=f"pj{qt}_{es}")
        for c in range(4):
            nc.tensor.matmul(scr, uhat[c][:, qt * 128:(qt + 1) * 128],
                             wp_sb[c][:, es * 512:(es + 1) * 512],
                             start=(c == 0), stop=(c == 3))
        ot = opool.tile([128, 512], F32, tag="out", name=f"ot{qt}_{es}")
        nc.vector.tensor_copy(out=ot, in_=scr)
        nc.sync.dma_start(out=y[qt * 128:(qt + 1) * 128, es * 512:(es + 1) * 512], in_=ot)

    # ---- attention state --------------------------------------------------
    v_emitted = [False] * 16
    gkt = [0]  # global kt counter -> stP rotation
    eS = {}  # (hidx, s, kt) -> exp'd scores [128, 1024] = [A | B]
    eacc = {}  # (hidx, s) -> accumulated E
    recs = {}  # (hidx, s) -> (recA, recB)
    passes = []  # pv pass FIFO: dicts

    def emit_scores_exps(hidx, p, h, kt):
        qk_q, qk_k = qkT[2 * p], qkT[2 * p + 1]
        g = gkt[0]
        gkt[0] += 1
        tiles = (stP[(2 * g) % 3], stP[(2 * g + 1) % 3])
        for s in range(2):  # chunk pair (A, B) for q-slice s
            q0 = h * 1024 + s * 512
            for head in range(2):
                r = slice(head * 64, head * 64 + 64)
                ch = tiles[s][:, head * 512:(head + 1) * 512]
                for lh in range(2):
                    k0 = kt * 128 + lh * 64
                    nc.tensor.matmul(ch[lh * 64:(lh + 1) * 64, :], qk_k[r, k0:k0 + 64],
                                     qk_q[r, q0:q0 + 512], start=True, stop=True)
        for s, pool in ((0, e0pool), (1, e1pool)):
            et = pool.tile([128, 1024], BF, tag=f"e{s}", name=f"e{hidx}_{s}_{kt}")
            nc.scalar.activation(out=et, in_=tiles[s][:, :], func=EXP, scale=SCALE)
            eS[(hidx, s, kt)] = et
            if kt == 0:
                ea = eapool.tile([128, 1024], BF, tag=f"ea{s}", name=f"ea{hidx}_{s}")
                eacc[(hidx, s)] = ea
                nc.vector.tensor_copy(out=ea, in_=et)
            else:
                nc.vector.tensor_add(out=eacc[(hidx, s)], in0=eacc[(hidx, s)], in1=et)

    def rec_chain(hidx, s):
        ea = eacc[(hidx, s)]
        sp = psW.tile([128, 512], F32, tag="scr", name=f"sums{hidx}_{s}")
        nc.tensor.matmul(sp[0:64, :], ones64, ea[:, 0:512], start=True, stop=True)
        nc.tensor.matmul(sp[64:128, :], ones64, ea[:, 512:1024], start=True, stop=True)
        ss = spool.tile([128, 512], F32, tag="sums", name=f"ss{hidx}_{s}")
        nc.vector.tensor_copy(out=ss, in_=sp)
        out = []
        for hb in (0, 1):
            rsp = spool.tile([128, 4], F32, tag="rsp", name=f"rsp{hidx}_{s}_{hb}")
            row = ss[hb * 64:hb * 64 + 1, :].rearrange("p (a b) -> p a b", a=128)
            nc.gpsimd.dma_start(out=rsp, in_=row)
            rspr = spool.tile([128, 4], BF, tag="rspr", name=f"rspr{hidx}_{s}_{hb}")
            with nc.allow_low_precision(reason="bf16 softmax denominators are within tolerance"):
                nc.vector.reciprocal(out=rspr, in_=rsp)
            rrow = spool.tile([1, 512], BF, tag="rrow", bufs=1, name=f"rrow{hidx}_{s}_{hb}")
            nc.gpsimd.dma_start(out=rrow[0:1, :].rearrange("p (a b) -> p a b", a=128), in_=rspr)
            rec = recpool.tile([128, 512], BF, tag="rec", name=f"rec{hidx}_{s}_{hb}")
            nc.gpsimd.partition_broadcast(out_ap=rec[:, :], in_ap=rrow[0:1, :])
            out.append(rec)
        recs[(hidx, s)] = out

    def emit_norm(ps):
        # drain ut immediately (frees the single PSUM bank), then normalize
        # in place in SBUF once the reciprocal broadcast lands.
        p, h, s, hidx = ps["p"], ps["h"], ps["s"], ps["hidx"]
        recA, recB = recs[(hidx, s)]
        ucols = slice(h * 1024 + s * 512, h * 1024 + (s + 1) * 512)
        nc.vector.tensor_copy(out=uhat[p][:, ucols], in_=ps["ut"])
        nc.vector.tensor_mul(uhat[p][0:64, ucols], uhat[p][0:64, ucols], recA[0:64, :])
        nc.vector.tensor_mul(uhat[p][64:128, ucols], uhat[p][64:128, ucols], recB[64:128, :])

    def pump_pv(cur_hidx, cur_step, budget):
        while budget > 0 and passes:
            ps = passes[0]
            if ps["kt"] >= 16:
                if recs.get((ps["hidx"], ps["s"])) is None:
                    break  # rec not emitted yet (same-half): wait for end block
                emit_norm(ps)
                passes.pop(0)
                continue
            kt = ps["kt"]
            if ps["hidx"] == cur_hidx and kt > cur_step - 2:
                break
            if not v_emitted[kt]:
                break
            if ps["ut"] is None:
                ps["ut"] = psU.tile([128, 512], F32, tag="ut", name=f"ut{ps['hidx']}_{ps['s']}")
            e = eS.pop((ps["hidx"], ps["s"], kt))
            p = ps["p"]
            st, sp = (kt == 0), (kt == 15)
            nc.tensor.matmul(ps["ut"][0:64, :], v_sb[kt][:, p * 128:p * 128 + 64], e[:, 0:512], start=st, stop=sp)
            nc.tensor.matmul(ps["ut"][64:128, :], v_sb[kt][:, p * 128 + 64:(p + 1) * 128], e[:, 512:1024], start=st, stop=sp)
            ps["kt"] += 1
            budget -= 1

    # ---- dense filler schedule -------------------------------------------
    QK = lambda f, t, a=False: (lambda: qk_chain(f, t, a))
    V = lambda t, a=False: (lambda: v_chain(t, a))
    PJ = lambda qt, es: (lambda: proj_chain(qt, es))
    fillers = [
        # p0h0: all v chains + K-p0 + pair-1 features; early chains
        # double-buffer through the still-idle ut bank, popped 2/step
        [V(2, True), QK(1, 1), V(3, True), QK(2, 0, True), V(4, True),
         QK(1, 2), V(5, True), QK(2, 1), V(6, True), QK(1, 3), V(7, True),
         QK(3, 0), V(8), V(9), V(10), V(11), V(12), V(13), V(14), V(15)],
        # p1h0
        [QK(3, 1), QK(3, 2), QK(3, 3), QK(4, 0), QK(4, 1), QK(5, 0),
         QK(5, 1)],
        # p2h0
        [QK(5, 2), QK(5, 3), QK(6, 0), QK(6, 1), QK(7, 0), QK(7, 1)],
        # p3h0
        [QK(7, 2), QK(7, 3), QK(0, 2), QK(0, 3), QK(2, 2), QK(2, 3)],
        # p0h1
        [QK(4, 2), QK(4, 3), QK(6, 2), QK(6, 3)],
        # p1h1 .. p3h1: proj for h0 q-tiles
        [PJ(qt, es) for qt in range(0, 3) for es in range(2)],
        [PJ(qt, es) for qt in range(3, 6) for es in range(2)],
        [PJ(qt, es) for qt in range(6, 8) for es in range(2)],
    ]

    # ---- lead-in ----------------------------------------------------------
    qk_chain(0, 0)
    qk_chain(0, 1, alt=True)
    qk_chain(1, 0)
    v_chain(0, alt=True)
    v_chain(1)

    # ---- main loop --------------------------------------------------------
    for hidx, (p, h) in enumerate(HALVES):
        fl = list(fillers[hidx])
        for kt in range(16):
            for _ in range(2 if len(fl) > 16 - kt else 1):
                if fl:
                    fl.pop(0)()
            pump_pv(hidx, kt, 6 if kt < 4 else 3)
            emit_scores_exps(hidx, p, h, kt)
            if kt == 8:
                passes.append({"hidx": hidx, "p": p, "h": h, "s": 0, "kt": 0, "ut": None})
            pump_pv(hidx, kt, 3)
        rec_chain(hidx, 0)
        rec_chain(hidx, 1)
        passes.append({"hidx": hidx, "p": p, "h": h, "s": 1, "kt": 0, "ut": None})

    # ---- tail -------------------------------------------------------------
    guard = 0
    while passes and guard < 200:
        pump_pv(-1, 99, 4)
        guard += 1
    for qt in range(8, 16):
        for es in range(2):
            proj_chain(qt, es)


_NC_CACHE = {}


def _build_nc():
    if "nc" in _NC_CACHE:
        return _NC_CACHE["nc"]
    nc = bacc.Bacc("TRN2", target_bir_lowering=False, debug=False, num_devices=N_CORES)
    xT = nc.dram_tensor("xT", [D, NT], BF, kind="ExternalInput").ap()
    wqk = nc.dram_tensor("wqk", [D, 1024], BF, kind="ExternalInput").ap()
    wv = nc.dram_tensor("wv", [D, 512], BF, kind="ExternalInput").ap()
    wp = nc.dram_tensor("wp", [512, 1024], BF, kind="ExternalInput").ap()
    y = nc.dram_tensor("y", [NT, 1024], F32, kind="ExternalOutput").ap()
    from contextlib import ExitStack

    with tile.TileContext(nc) as tc, ExitStack() as ctx:
        _body(tc, ctx, y, xT, wqk, wv, wp)
    nc.compile()
    _NC_CACHE["nc"] = nc
    return nc


def _prepare_in_maps(x, W_qkv, W_proj):
    x = np.asarray(x, dtype=np.float32)
    W_qkv = np.asarray(W_qkv, dtype=np.float32)
    W_proj = np.asarray(W_proj, dtype=np.float32)
    in_maps = []
    for c in range(N_CORES):
        b, hg = divmod(c, 2)
        cs = slice(hg * 512, (hg + 1) * 512)
        xTc = np.ascontiguousarray(x[b].T).astype(BF16)
        Qc = W_qkv[:, 0:1024][:, cs]
        Kc = W_qkv[:, 1024:2048][:, cs]
        # per-pair interleave: [Q-pair0 | K-pair0 | Q-pair1 | K-pair1 | ...]
        blocks = []
        for p in range(4):
            blocks.append(Qc[:, p * 128:(p + 1) * 128])
            blocks.append(Kc[:, p * 128:(p + 1) * 128])
        wqk = np.ascontiguousarray(np.concatenate(blocks, axis=1)).astype(BF16)
        wv = np.ascontiguousarray(W_qkv[:, 2048:3072][:, cs]).astype(BF16)
        wp = np.ascontiguousarray(W_proj[cs, :]).astype(BF16)
        in_maps.append({"xT": xTc, "wqk": wqk, "wv": wv, "wp": wp})
    return in_maps


def _run(x, W_qkv, W_proj, b_proj, trace=False):
    nc = _build_nc()
    in_maps = _prepare_in_maps(x, W_qkv, W_proj)
    res = bass_utils.run_bass_kernel_spmd(
        nc, in_maps, core_ids=list(range(N_CORES)), trace=trace
    )
    b_proj = np.asarray(b_proj, dtype=np.float32)
    y = np.empty((4, NT, D), dtype=np.float32)
    for b in range(4):
        y[b] = res.results[2 * b]["y"] + res.results[2 * b + 1]["y"] + b_proj[None, :]
    return y, res


def kernel(x, W_qkv, W_proj, b_proj):
    y, _ = _run(x, W_qkv, W_proj, b_proj, trace=False)
    return y


# revision 18
# speedup vs baseline: 1.8472x; 1.2213x over previous
"""Trainium2 Bass kernel for nn_Attention (B=4, N=2048, D=1024, H=16, Hd=64).

Sharding: 8 cores = 4 batches x 2 head-groups. Core c handles batch c//2 and
heads [ (c%2)*8, (c%2)*8+8 ).  Each core computes qkv projections for its
heads, attention, and a partial output projection (contraction over its 512
head-dims of W_proj). Host sums the two partials per batch and adds b_proj.

Per-core kernel (all matmuls bf16 with fp32 PSUM accumulation):
  - qkT[f, t]  = sum_d Wqk[d, f] * xT[d, t]     (Q^T/K^T per head, [64, 2048])
  - v[t, f]    = sum_d xT[d, t] * Wv[d, f]       ([2048, 512], keys-major)
  - per head pair (2 heads packed in PE row/col groups):
      S^T[k, q] = sum_d K^T[d, k] Q^T[d, q]      (keys on partitions)
      E = exp(S^T / 8)   (ScalarE, bf16 out)
      U^T[hd, q] += sum_k V[k, hd] E[k, q]       (PSUM accumulate over key tiles)
      Eacc += E (VectorE);  sums = partition_all_reduce(Eacc)  (GpSimd)
      Uhat = U^T * (1/sums)                      (normalize during PSUM drain)
  - y[q, e] = sum_hd Uhat[hd, q] Wp[hd, e]       (partial; host adds pair+bias)
"""

import os
import sys
import types

import numpy as np

# --- environment bootstrap (grading env == dev env: axon-tunneled trn2) ----
for _p in ("/opt/trn_rl_repo", "/root/.axon_site/_ro/trn_rl_repo"):
    if _p not in sys.path and os.path.isdir(_p):
        sys.path.append(_p)

import ml_dtypes  # noqa: E402

BF16 = ml_dtypes.bfloat16


def _install_ntff_shim():
    """antenv.axon_hooks is missing on this image; provide it and register the
    ctypes NTFF hook so trace=True can report HW exec time."""
    if "antenv.axon_hooks" in sys.modules:
        return
    mod = types.ModuleType("antenv.axon_hooks")
    mod._hook = None
    mod.set_axon_ntff_profile_hook = lambda h: setattr(mod, "_hook", h)
    mod.get_axon_ntff_profile_hook = lambda: mod._hook
    sys.modules["antenv.axon_hooks"] = mod
    try:
        import antenv

        antenv.axon_hooks = mod
    except ImportError:
        pass
    try:
        from trn_agent_boot.trn_boot import _ntff_profile_via_ctypes

        hook = _ntff_profile_via_ctypes("/opt/axon/libaxon_pjrt.so")
        if hook is not None:
            mod.set_axon_ntff_profile_hook(hook)
    except Exception:
        pass


_install_ntff_shim()

import concourse.bacc as bacc  # noqa: E402
import concourse.bass as bass  # noqa: E402
import concourse.tile as tile  # noqa: E402
import concourse.bass_isa as bass_isa  # noqa: E402
from concourse import mybir  # noqa: E402
import concourse.bass_utils as bass_utils  # noqa: E402

# no S3 in the container; keep NTFF artifacts local
bass_utils.upload_artifacts = lambda tmpdir: tmpdir

F32 = mybir.dt.float32
BF = mybir.dt.bfloat16
EXP = mybir.ActivationFunctionType.Exp

N_CORES = 8
NT = 2048  # tokens
D = 1024  # d_model
NH_LOC = 8  # heads per core
HD = 64  # head dim
SCALE = HD**-0.5


def _body(tc: "tile.TileContext", ctx, y, xT, wqk, wv, wp):
    nc = tc.nc

    wpool = ctx.enter_context(tc.tile_pool(name="wpool", bufs=1))
    qkpool = ctx.enter_context(tc.tile_pool(name="qkpool", bufs=1))
    vpool = ctx.enter_context(tc.tile_pool(name="vpool", bufs=1))
    upool = ctx.enter_context(tc.tile_pool(name="upool", bufs=1))
    epool = ctx.enter_context(tc.tile_pool(name="epool", bufs=10))
    eaccpool = ctx.enter_context(tc.tile_pool(name="eaccpool", bufs=4))
    spool = ctx.enter_context(tc.tile_pool(name="spool", bufs=1))
    rpool = ctx.enter_context(tc.tile_pool(name="rpool", bufs=1))
    opool = ctx.enter_context(tc.tile_pool(name="opool", bufs=3))
    # PSUM budget (8 banks): scores 2x[128,1024] (4) + U^T/sums [128,1024]
    # (2) + filler pool 2x[128,512] (2). The filler pool decouples qkv/proj
    # background matmuls from the score/exp pipeline slots.
    psb = ctx.enter_context(tc.tile_pool(name="psb", bufs=2, space="PSUM"))
    psu = ctx.enter_context(tc.tile_pool(name="psu", bufs=1, space="PSUM"))
    pfill = ctx.enter_context(tc.tile_pool(name="pfill", bufs=2, space="PSUM"))

    # ---- persistent SBUF tensors -----------------------------------------
    xT_sb = [wpool.tile([128, NT], BF, tag=f"xT{i}", name=f"xT{i}") for i in range(8)]
    wqk_sb = [wpool.tile([128, 1024], BF, tag=f"wqk{i}", name=f"wqk{i}") for i in range(8)]
    wv_sb = [wpool.tile([128, 512], BF, tag=f"wv{i}", name=f"wv{i}") for i in range(8)]
    wp_sb = [wpool.tile([128, 1024], BF, tag=f"wp{i}", name=f"wp{i}") for i in range(4)]
    # need-ordered input DMAs: the lead-in chains consume x tokens 0:1024 and
    # the Q/K weight halves first; later token chunks and wp can trickle in.
    for i in range(8):
        nc.sync.dma_start(out=xT_sb[i][:, 0:512], in_=xT[i * 128:(i + 1) * 128, 0:512])
    for i in range(8):
        nc.sync.dma_start(out=wqk_sb[i][:, 0:512], in_=wqk[i * 128:(i + 1) * 128, 0:512])
    for i in range(8):
        nc.sync.dma_start(out=wqk_sb[i][:, 512:1024], in_=wqk[i * 128:(i + 1) * 128, 512:1024])
    for i in range(8):
        nc.sync.dma_start(out=xT_sb[i][:, 512:1024], in_=xT[i * 128:(i + 1) * 128, 512:1024])
    for i in range(8):
        nc.sync.dma_start(out=wv_sb[i], in_=wv[i * 128:(i + 1) * 128, :])
    for ts in (2, 3):
        for i in range(8):
            nc.sync.dma_start(out=xT_sb[i][:, ts * 512:(ts + 1) * 512],
                              in_=xT[i * 128:(i + 1) * 128, ts * 512:(ts + 1) * 512])
    for i in range(4):
        nc.sync.dma_start(out=wp_sb[i], in_=wp[i * 128:(i + 1) * 128, :])

    qkT = [qkpool.tile([128, NT], BF, tag=f"qkT{f}", name=f"qkT{f}") for f in range(8)]
    v_sb = [vpool.tile([128, 512], BF, tag=f"v{t}", name=f"v{t}") for t in range(16)]
    uhat = [upool.tile([128, NT], BF, tag=f"uh{p}", name=f"uh{p}") for p in range(4)]
    ones64 = wpool.tile([128, 64], BF, tag="ones64", name="ones64")
    nc.vector.memset(ones64, 1.0)

    # ---- background units (run on the filler PSUM pool) -------------------
    def qk_sub(f, ts2):
        # qkT[f][:, ts2*512:(ts2+1)*512] = (x @ Wqk[:, f-chunk]).T slice
        ps = pfill.tile([128, 512], F32, tag="pf", name=f"qk_ps{f}_{ts2}")
        for d in range(8):
            nc.tensor.matmul(
                ps[:, :],
                wqk_sb[d][:, f * 128 : (f + 1) * 128],
                xT_sb[d][:, ts2 * 512 : (ts2 + 1) * 512],
                start=(d == 0),
                stop=(d == 7),
            )
        nc.vector.tensor_copy(out=qkT[f][:, ts2 * 512 : (ts2 + 1) * 512], in_=ps[:])

    def v_unit(t):
        ps = pfill.tile([128, 512], F32, tag="pf", name=f"v_ps{t}")
        for d in range(8):
            nc.tensor.matmul(
                ps[:, :],
                xT_sb[d][:, t * 128 : (t + 1) * 128],
                wv_sb[d][:, :],
                start=(d == 0),
                stop=(d == 7),
            )
        nc.vector.tensor_copy(out=v_sb[t], in_=ps[:])

    def proj_sub(qt, es, pool=None, tag="pf"):
        # y[qt-tile, es-slice] partial over this core's 512 head dims
        pj = (pool or pfill).tile([128, 512], F32, tag=tag, name=f"pj{qt}_{es}")
        for c in range(4):
            nc.tensor.matmul(
                pj[:, :],
                uhat[c][:, qt * 128 : (qt + 1) * 128],
                wp_sb[c][:, es * 512 : (es + 1) * 512],
                start=(c == 0),
                stop=(c == 3),
            )
        ot = opool.tile([128, 512], F32, tag="out", name=f"ot{qt}_{es}")
        nc.vector.tensor_copy(out=ot, in_=pj[:])
        nc.sync.dma_start(
            out=y[qt * 128 : (qt + 1) * 128, es * 512 : (es + 1) * 512], in_=ot
        )

    # ---- attention for one pair of heads, one query half ------------------
    # `fillers`: background units woven one-per-kt-step into this pair's
    # stream. Every filler MUST be emitted before the first instruction that
    # consumes its output (in-order engine queues deadlock otherwise), so
    # each list is fully drained inside its own pair-half (15 slots >= len).
    def attention_pair_half(p, half, fillers=()):
        fillers = list(fillers)
        assert len(fillers) <= 15
        A, B = 2 * p, 2 * p + 1
        hsl = slice(half * 1024, (half + 1) * 1024)
        qA = qkT[p][0:64, hsl]
        qB = qkT[p][64:128, hsl]
        kA = qkT[4 + p][0:64, :]
        kB = qkT[4 + p][64:128, :]
        ut = psu.tile([128, 1024], F32, tag="ut", name=f"ut{p}_{half}")
        eaccA = eaccpool.tile([128, 1024], BF, tag="eacc", name=f"eaccA{p}_{half}")
        eaccB = eaccpool.tile([128, 1024], BF, tag="eacc", name=f"eaccB{p}_{half}")
        eAs, eBs = [], []

        def pv(ktpv, hb, s):
            e, head, r0 = (eAs[ktpv], A, 0) if hb == 0 else (eBs[ktpv], B, 64)
            ssl = slice(s * 512, (s + 1) * 512)
            nc.tensor.matmul(
                ut[r0 : r0 + 64, ssl],
                v_sb[ktpv][:, head * 64 : (head + 1) * 64],
                e[:, ssl],
                start=(ktpv == 0),
                stop=(ktpv == 15),
            )

        def eacc_step(ktpv):
            if ktpv == 0:
                nc.vector.tensor_copy(out=eaccA, in_=eAs[0])
                nc.vector.tensor_copy(out=eaccB, in_=eBs[0])
            else:
                nc.vector.tensor_add(out=eaccA, in0=eaccA, in1=eAs[ktpv])
                nc.vector.tensor_add(out=eaccB, in0=eaccB, in1=eBs[ktpv])

        for kt in range(16):
            ksl = slice(kt * 128, (kt + 1) * 128)
            eA = epool.tile([128, 1024], BF, tag="e", name=f"eA{p}_{half}_{kt}")
            eB = epool.tile([128, 1024], BF, tag="e", name=f"eB{p}_{half}_{kt}")
            eAs.append(eA)
            eBs.append(eB)
            # ready work (PV for kt-1, eacc, filler) goes BEFORE the QK score
            # groups: the in-order PE queue then reaches the QK slot-waits with
            # the previous exps already retired, instead of stalling on them.
            if kt > 1:
                pv(kt - 2, 0, 0)
                pv(kt - 2, 1, 0)
                pv(kt - 2, 0, 1)
                pv(kt - 2, 1, 1)
                eacc_step(kt - 2)
            if kt > 0 and fillers:
                fillers.pop(0)()
            # A/B interleaved: adjacent matmuls hit disjoint PE row groups,
            # so they overlap in the array (the pv(kt-2) lag guarantees both
            # score slots are already free when the PE reaches this block).
            stA = psb.tile([128, 1024], F32, tag="psb", name=f"stA{p}_{half}_{kt}")
            stB = psb.tile([128, 1024], F32, tag="psb", name=f"stB{p}_{half}_{kt}")
            for s in range(2):
                q0 = half * 1024 + s * 512
                nc.tensor.matmul(
                    stA[:, s * 512 : (s + 1) * 512], kA[:, ksl], qkT[p][0:64, q0 : q0 + 512],
                    start=True, stop=True,
                )
                nc.tensor.matmul(
                    stB[:, s * 512 : (s + 1) * 512], kB[:, ksl], qkT[p][64:128, q0 : q0 + 512],
                    start=True, stop=True,
                )
            nc.scalar.activation(out=eA[:], in_=stA[:], func=EXP, scale=SCALE)
            nc.scalar.activation(out=eB[:], in_=stB[:], func=EXP, scale=SCALE)
        while fillers:
            fillers.pop(0)()
        for ktl in (14, 15):
            for s in range(2):
                pv(ktl, 0, s)
                pv(ktl, 1, s)
            eacc_step(ktl)
        # drain U^T (unnormalized) so the PSUM accumulator frees quickly
        nc.vector.tensor_copy(out=uhat[p][:, hsl], in_=ut[:])
        # softmax denominators for this half (ones-matmul -> spread-recip ->
        # partition_broadcast), then normalize in place
        sums_ps = psu.tile([128, 1024], F32, tag="ut", name=f"sums_ps{p}_{half}")
        for s in range(2):
            ssl = slice(s * 512, (s + 1) * 512)
            nc.tensor.matmul(sums_ps[0:64, ssl], ones64[:], eaccA[:, ssl], start=True, stop=True)
            nc.tensor.matmul(sums_ps[64:128, ssl], ones64[:], eaccB[:, ssl], start=True, stop=True)
        sums = spool.tile([128, 1024], F32, tag="sums", name=f"sums{p}_{half}")
        nc.vector.tensor_copy(out=sums, in_=sums_ps[:])
        for hb in (0, 1):
            r0 = hb * 64
            rsp = spool.tile([128, 8], F32, tag="rsp", name=f"rsp{p}_{half}_{hb}")
            row = sums[r0 : r0 + 1, :].rearrange("p (a b) -> p a b", a=128)
            nc.gpsimd.dma_start(out=rsp[:], in_=row)
            rspr = spool.tile([128, 8], F32, tag="rspr", name=f"rspr{p}_{half}_{hb}")
            nc.vector.reciprocal(out=rspr[:], in_=rsp[:])
            rrow = spool.tile([1, 1024], F32, tag="rrow", bufs=1, name=f"rrow{p}_{half}_{hb}")
            nc.gpsimd.dma_start(
                out=rrow[0:1, :].rearrange("p (a b) -> p a b", a=128), in_=rspr[:]
            )
            rec = rpool.tile([128, 1024], F32, tag=f"rec{hb}", name=f"rec{p}_{half}_{hb}")
            nc.gpsimd.partition_broadcast(out_ap=rec[:, :], in_ap=rrow[0:1, :])
            nc.vector.tensor_mul(
                uhat[p][r0 : r0 + 64, hsl], uhat[p][r0 : r0 + 64, hsl], rec[r0 : r0 + 64, :]
            )

    # ---- schedule ---------------------------------------------------------
    # lead-in: q/k features for pair 0 plus the first v tiles; the rest of
    # the qkv projections and half-0's output projection weave into the
    # attention stream as per-pair filler lists (dependency-safe: each list
    # drains before the pair that consumes its outputs starts).
    def mk(fn, *args):
        return lambda: fn(*args)

    # minimal lead: pair-0 half-0 needs only Q half-0 (f0 ts0/1) and the
    # first key quarter (f4 ts0); the rest of f4 weaves in ahead of its kt
    # deadlines (f4tsX covers keys for kt in [4X, 4X+4), used at step 4X).
    qk_sub(0, 0)
    qk_sub(0, 1)
    qk_sub(4, 0)
    for t in range(7):
        v_unit(t)
    half0_fills = [
        [mk(v_unit, 7), mk(qk_sub, 4, 1), mk(v_unit, 8), mk(v_unit, 9),
         mk(qk_sub, 4, 2), mk(v_unit, 10), mk(v_unit, 11), mk(qk_sub, 4, 3),
         mk(v_unit, 12), mk(v_unit, 13), mk(v_unit, 14), mk(v_unit, 15),
         mk(qk_sub, 1, 0), mk(qk_sub, 1, 1), mk(qk_sub, 5, 0)],
        [mk(qk_sub, 5, 1), mk(qk_sub, 5, 2), mk(qk_sub, 5, 3),
         mk(qk_sub, 0, 2), mk(qk_sub, 0, 3), mk(qk_sub, 1, 2), mk(qk_sub, 1, 3)]
        + [mk(qk_sub, f, ts2) for f in (2, 6) for ts2 in range(4)],
        [mk(qk_sub, f, ts2) for f in (3, 7) for ts2 in range(4)],
        [],
    ]
    for p in range(4):
        attention_pair_half(p, 0, half0_fills[p])
    half1_fills = [
        [mk(proj_sub, qt, es) for qt in range(0, 2) for es in range(2)],
        [mk(proj_sub, qt, es) for qt in range(2, 4) for es in range(2)],
        [mk(proj_sub, qt, es) for qt in range(4, 6) for es in range(2)],
        [mk(proj_sub, qt, es) for qt in range(6, 8) for es in range(2)],
    ]
    for p in range(4):
        attention_pair_half(p, 1, half1_fills[p])
    for qt in range(8, 16):
        for es in range(2):
            if (qt * 2 + es) % 2 == 0:
                proj_sub(qt, es)
            else:
                proj_sub(qt, es, pool=psb, tag="psb")


_NC_CACHE = {}


def _build_nc():
    if "nc" in _NC_CACHE:
        return _NC_CACHE["nc"]
    nc = bacc.Bacc("TRN2", target_bir_lowering=False, debug=False, num_devices=N_CORES)
    xT = nc.dram_tensor("xT", [D, NT], BF, kind="ExternalInput").ap()
    wqk = nc.dram_tensor("wqk", [D, 1024], BF, kind="ExternalInput").ap()
    wv = nc.dram_tensor("wv", [D, 512], BF, kind="ExternalInput").ap()
    wp = nc.dram_tensor("wp", [512, 1024], BF, kind="ExternalInput").ap()
    y = nc.dram_tensor("y", [NT, 1024], F32, kind="ExternalOutput").ap()
    from contextlib import ExitStack

    with tile.TileContext(nc) as tc, ExitStack() as ctx:
        _body(tc, ctx, y, xT, wqk, wv, wp)
    nc.compile()
    _NC_CACHE["nc"] = nc
    return nc


def _prepare_in_maps(x, W_qkv, W_proj):
    x = np.asarray(x, dtype=np.float32)
    W_qkv = np.asarray(W_qkv, dtype=np.float32)
    W_proj = np.asarray(W_proj, dtype=np.float32)
    in_maps = []
    for c in range(N_CORES):
        b, hg = divmod(c, 2)
        cs = slice(hg * 512, (hg + 1) * 512)
        xTc = np.ascontiguousarray(x[b].T).astype(BF16)
        wqk = np.ascontiguousarray(
            np.concatenate([W_qkv[:, 0:1024][:, cs], W_qkv[:, 1024:2048][:, cs]], axis=1)
        ).astype(BF16)
        wv = np.ascontiguousarray(W_qkv[:, 2048:3072][:, cs]).astype(BF16)
        wp = np.ascontiguousarray(W_proj[cs, :]).astype(BF16)
        in_maps.append({"xT": xTc, "wqk": wqk, "wv": wv, "wp": wp})
    return in_maps


def _run(x, W_qkv, W_proj, b_proj, trace=False):
    nc = _build_nc()
    in_maps = _prepare_in_maps(x, W_qkv, W_proj)
    res = bass_utils.run_bass_kernel_spmd(
        nc, in_maps, core_ids=list(range(N_CORES)), trace=trace
    )
    b_proj = np.asarray(b_proj, dtype=np.float32)
    y = np.empty((4, NT, D), dtype=np.float32)
    for b in range(4):
        y[b] = res.results[2 * b]["y"] + res.results[2 * b + 1]["y"] + b_proj[None, :]
    return y, res


def kernel(x, W_qkv, W_proj, b_proj):
    y, _ = _run(x, W_qkv, W_proj, b_proj, trace=False)
    return y

